# revision 1
# baseline (speedup 1.0000x reference)
# Trainium2 Bass kernel for nn_AttentionWithMoLE — 8-core tensor-parallel over heads.
#
# Sharding: core c owns q-heads {2c, 2c+1} (wq cols 128c:128c+128), kv-head c
# (wk/wv cols 64c:64c+64), wo rows 128c:128c+128. Host sums the 8 partial outputs.
#
# MoLE algebra used on device (validated against the reference in numpy):
#   xq = 2*base + sum_e gate_e * lora_e          (softmax gates sum to 1)
#   logits[b,e'] = sum_s r[b,s]*(P1[b,s,e'] - mu[b,s]*C1[s,e']) + C2[e']  (then /tau)
#   P1[b,s,e'] = sum_f E_raw[b,s,f] * (g (.) We)[s,f,e']   (feature-sharded, AllReduce'd)
# Stats (sum, sumsq) and P1 partials are AllReduce'd across cores (288KB), gates
# computed replicated, then pass B builds q/k/v + RoPE + causal attention
# (scoresT orientation, exp without max-subtraction, bf16 probs, fused rowsum
# via a ones-column appended to V) + output projection.
import sys
import numpy as np

sys.path.insert(0, '/opt/trn_rl_repo')

import concourse.bass as bass
import concourse.bacc as bacc
import concourse.tile as tile
import concourse.mybir as mybir
from concourse.masks import make_identity

NC = 8
B, S, D, H, KVH, HD, NE, R = 4, 1024, 1024, 16, 8, 64, 4, 16
SCALING, GEPS = 2.0, 1e-6
BS = B * S
NST = S // 128           # 8 s-tiles
NT = NST * B             # 32 token tiles of 128
F16 = mybir.dt.float16
BF16 = mybir.dt.bfloat16
F32 = mybir.dt.float32
AX = mybir.AxisListType
OP = mybir.AluOpType
AF = mybir.ActivationFunctionType

_CACHE = {}
DEBUG = False
PHASES = 5
APARTS = 7


def _bcast_ap(ap, ins):
    """Insert step-0 broadcast dims: ins = list of (pos, count) into ap.ap."""
    dims = [list(d) for d in ap.ap]
    for pos, count in ins:
        dims.insert(pos, [0, count])
    return bass.AP(tensor=ap.tensor, offset=ap.offset, ap=dims)


def _emit(nc, tc, ctx, rep):
    g = nc._kernel_io  # dict of dram handles
    sfx = f"_r{rep}"

    # ---------------- resident pools ----------------
    res = ctx.enter_context(tc.tile_pool(name="res" + sfx, bufs=1))
    base_all = res.tile([128, NST, B, 256], F32)
    hqk_all = res.tile([128, NT, 128], F16)     # [er(q0:64,k64:128), t, tok]
    hv_all = res.tile([64, NT, 128], F16)
    qT_all = res.tile([64, NST, B, 2, 128], F16)  # [hd, st, b, head, tok] (prescaled 1/8)
    kT_all = res.tile([64, NST, B, 128], F16)
    vaug_all = res.tile([128, NST, B, 65], BF16)
    att_all = res.tile([128, NT, 128], F16)     # [tok(sq), t, o(2 heads)]
    stats_all = res.tile([128, NST, B, 18], F32)
    stats_ar = res.tile([128, NST, B, 18], F32)
    wqkv_sb = res.tile([128, 8, 256], F16)
    acat_sb = res.tile([128, 8, 192], F16)
    blq_sb = res.tile([64, 512], F16)
    blk_sb = res.tile([128, 256], F16)
    blv_sb = res.tile([64, 256], F16)
    bbqk_sb = res.tile([128, 256], F16)
    bbv_sb = res.tile([64, 256], F16)
    wo_sb = res.tile([128, 1024], F16)
    cos_sb = res.tile([128, NST, 64], F32)
    sin_sb = res.tile([128, NST, 64], F32)
    c1_sb = res.tile([128, NST, 12], F32)
    c2it_sb = res.tile([1, 48], F32)
    itau_sb = res.tile([128, 3], F32)
    maskt_sb = res.tile([128, 128], F32)
    ident_sb = res.tile([128, 128], F16)
    ones_sb = res.tile([128, 1], F32)
    geps_sb = res.tile([128, 1], F32)
    gvqk_sb = res.tile([128, B], F32)
    gvv_sb = res.tile([64, B], F32)
    gates_sb = res.tile([1, 48], F32)

    dma = nc.sync.dma_start
    dma(out=wqkv_sb[:], in_=g['wqkv16'][:].rearrange('c p o -> p c o'))
    dma(out=acat_sb[:], in_=g['acat16'][:].rearrange('c p o -> p c o'))
    dma(out=blq_sb[:], in_=g['blq16'][:])
    dma(out=blk_sb[64:128, :], in_=g['blk16'][:])
    dma(out=blv_sb[:], in_=g['blv16'][:])
    dma(out=bbqk_sb[:], in_=g['bbqk16'][:])
    dma(out=bbv_sb[:], in_=g['bbv16'][:])
    dma(out=wo_sb[:], in_=g['wo16'][:])
    dma(out=cos_sb[:], in_=g['cosr'][:].rearrange('(st p) i -> p st i', p=128))
    dma(out=sin_sb[:], in_=g['sinr'][:].rearrange('(st p) i -> p st i', p=128))
    dma(out=c1_sb[:], in_=g['c1_12'][:].rearrange('(st p) f -> p st f', p=128))
    c2 = g['c2it48'][:]
    dma(out=c2it_sb[:], in_=bass.AP(tensor=c2.tensor, offset=c2.offset,
                                    ap=[[0, 1], [1, 48]]))
    it = g['itau3'][:]
    dma(out=itau_sb[:], in_=bass.AP(tensor=it.tensor, offset=it.offset,
                                    ap=[[0, 128], [1, 3]]))
    dma(out=maskt_sb[:], in_=g['maskt'][:])
    make_identity(nc, ident_sb[:])
    nc.vector.memset(ones_sb[:], 1.0)
    nc.vector.memset(geps_sb[:], GEPS)
    nc.vector.memset(vaug_all[:, :, :, 64:65], 1.0)

    # ---------------- phase A ----------------
    if PHASES < 1:
        return
    with tc.tile_pool(name="pa" + sfx, bufs=2) as pa, \
         tc.tile_pool(name="pa1" + sfx, bufs=1) as pa1, \
         tc.tile_pool(name="ppb" + sfx, bufs=2, space="PSUM") as ppb, \
         tc.tile_pool(name="pph" + sfx, bufs=2, space="PSUM") as pph, \
         tc.tile_pool(name="ppl" + sfx, bufs=1, space="PSUM") as ppl:
        for st in range(NST):
            wg_t = pa.tile([128, 4, 1024], F16, tag="wg")
            dma(out=wg_t[:],
                in_=g['wg16'][:].rearrange('(st p) e f -> p st e f', p=128)[:, st])
            for b in range(B):
                t = st * B + b
                tok0 = b * S + st * 128
                xt = pa.tile([128, 8, 128], F16, tag="xt")
                dma(out=xt[:],
                    in_=g['xt16'][:].rearrange('c p t -> p c t')[:, :, tok0:tok0 + 128])
                # base[tok, 256] and hT[er, tok]
                pbase = ppb.tile([128, 256], F32, tag="pbase")
                phqk = pph.tile([128, 128], F32, tag="phqk")
                phv = pph.tile([64, 128], F32, tag="phv")
                for ch in range(8):
                    nc.tensor.matmul(pbase[:], xt[:, ch, :], wqkv_sb[:, ch, :],
                                     start=(ch == 0), stop=(ch == 7))
                    nc.tensor.matmul(phqk[:], acat_sb[:, ch, 0:128],
                                     xt[:, ch, :], start=(ch == 0), stop=(ch == 7))
                    nc.tensor.matmul(phv[:], acat_sb[:, ch, 128:192],
                                     xt[:, ch, :], start=(ch == 0), stop=(ch == 7))
                nc.scalar.copy(base_all[:, st, b, :], pbase[:])
                b16 = pa.tile([128, 256], F16, tag="b16")
                nc.scalar.copy(b16[:], pbase[:])
                nc.vector.tensor_copy(hqk_all[:, t, :], phqk[:])
                nc.vector.tensor_copy(hv_all[:, t, :], phv[:])
                # lora_e via block-diag B' (K=64)
                if APARTS < 2:
                    continue
                pl = ppl.tile([128, 1024], F32, tag="pl")
                nc.tensor.matmul(pl[:, 0:512], hqk_all[0:64, t, :], blq_sb[:])
                nc.tensor.matmul(pl[:, 512:768], hqk_all[64:128, t, :], blk_sb[64:128, :])
                nc.tensor.matmul(pl[:, 768:1024], hv_all[:, t, :], blv_sb[:])
                # Ecat per proj + stats + P1
                if APARTS < 4:
                    continue
                brep = pa.tile([128, 1024], F16, tag="brep")
                bv = b16[:]
                nc.sync.dma_start(out=brep[:, 0:512].rearrange('p (e o) -> p e o', e=4),
                    in_=bass.AP(tensor=bv.tensor, offset=bv.offset,
                                ap=[list(bv.ap[0]), [0, 4], [1, 128]]))
                nc.sync.dma_start(out=brep[:, 512:768].rearrange('p (e o) -> p e o', e=4),
                    in_=bass.AP(tensor=bv.tensor, offset=bv.offset + 128,
                                ap=[list(bv.ap[0]), [0, 4], [1, 64]]))
                nc.sync.dma_start(out=brep[:, 768:1024].rearrange('p (e o) -> p e o', e=4),
                    in_=bass.AP(tensor=bv.tensor, offset=bv.offset + 192,
                                ap=[list(bv.ap[0]), [0, 4], [1, 64]]))
                ecat = pa.tile([128, 1024], F16, tag="ecat")
                junk = pa1.tile([128, 512], F16, tag="junk")
                sa = stats_all
                specs = [(0, 512, 0, 128, 0), (512, 768, 128, 192, 2),
                         (768, 1024, 192, 256, 4)]
                for (f0, f1, c0, c1v, sidx) in specs:
                    nf = f1 - f0
                    ev = ecat[:, f0:f1]
                    nc.vector.tensor_add(ev, pl[:, f0:f1], brep[:, f0:f1])
                    nc.vector.tensor_reduce(sa[:, st, b, sidx:sidx + 1],
                                            ev, axis=AX.X, op=OP.add)
                    nc.vector.tensor_mul(junk[:, 0:nf], ev, ev)
                    nc.vector.tensor_reduce(sa[:, st, b, sidx + 1:sidx + 2],
                                            junk[:, 0:nf], axis=AX.X, op=OP.add)
                for pi, (f0, f1) in enumerate([(0, 512), (512, 768), (768, 1024)]):
                    nf = f1 - f0
                    for e in range(4):
                        nc.vector.tensor_mul(junk[:, 0:nf], ecat[:, f0:f1],
                                             wg_t[:, e, f0:f1])
                        nc.vector.tensor_reduce(
                            sa[:, st, b, 6 + 4 * pi + e:7 + 4 * pi + e],
                            junk[:, 0:nf], axis=AX.X, op=OP.add)

    if PHASES < 2:
        return
    # ---------------- AllReduce ----------------
    with tc.tile_pool(name="dr" + sfx, bufs=1, space="DRAM") as dr:
        ar_in = dr.tile([128, NST, B, 18], F32)
        ar_out = dr.tile([128, NST, B, 18], F32)
        dma(out=ar_in[:], in_=stats_all[:])
        nc.gpsimd.collective_compute(
            "AllReduce", OP.add, replica_groups=[list(range(NC))],
            ins=[ar_in[:]], outs=[ar_out[:]])
        dma(out=stats_ar[:], in_=ar_out[:])

    if PHASES < 3:
        return
    # ---------------- gates (replicated) ----------------
    with tc.tile_pool(name="pg" + sfx, bufs=1) as pg, \
         tc.tile_pool(name="pgp" + sfx, bufs=1, space="PSUM") as pgp:
        lpart = pg.tile([128, 48], F32)
        for pi, Ff in enumerate([NE * H * HD, NE * KVH * HD, NE * KVH * HD]):
            s1 = stats_ar[:, :, :, 2 * pi if pi == 0 else 2 * pi]
            s1 = stats_ar[:, :, :, 2 * pi]
            s2 = stats_ar[:, :, :, 2 * pi + 1]
            mu = pg.tile([128, NST, B], F32, tag=f"mu{pi}")
            vr = pg.tile([128, NST, B], F32, tag=f"vr{pi}")
            rr = pg.tile([128, NST, B], F32, tag=f"rr{pi}")
            rm = pg.tile([128, NST, B], F32, tag=f"rm{pi}")
            t1 = pg.tile([128, NST, B], F32, tag=f"t1{pi}")
            t2 = pg.tile([128, NST, B], F32, tag=f"t2{pi}")
            nc.vector.tensor_scalar_mul(mu[:], s1, 1.0 / Ff)
            nc.vector.tensor_scalar_mul(vr[:], s2, 1.0 / Ff)
            nc.vector.tensor_mul(t1[:], mu[:], mu[:])
            nc.vector.tensor_sub(vr[:], vr[:], t1[:])
            nc.scalar.activation(out=rr[:], in_=vr[:], func=AF.Sqrt, bias=geps_sb[:],
                                 scale=1.0)
            nc.vector.reciprocal(rr[:], rr[:])
            nc.vector.tensor_scalar_mul(rr[:], rr[:], itau_sb[:, pi:pi + 1])
            nc.vector.tensor_mul(rm[:], rr[:], mu[:])
            for e in range(4):
                p1 = stats_ar[:, :, :, 6 + 4 * pi + e]
                c1col = c1_sb[:, :, 4 * pi + e]
                nc.vector.tensor_mul(t1[:], p1, rr[:])
                for bq in range(B):
                    nc.vector.tensor_mul(t2[:, :, bq], c1col, rm[:, :, bq])
                nc.vector.tensor_sub(t1[:], t1[:], t2[:])
                # reduce over st keeping b -> columns p*16 + b*4 + e
                src = bass.AP(tensor=t1.tensor, offset=t1[:].offset,
                              ap=[list(t1[:].ap[0]), [1, B], [B, NST]])
                dst = bass.AP(tensor=lpart.tensor,
                              offset=lpart[:].offset + 16 * pi + e,
                              ap=[list(lpart[:].ap[0]), [4, B]])
                nc.vector.tensor_reduce(dst, src, axis=AX.X, op=OP.add)
        pl48 = pgp.tile([1, 48], F32)
        nc.tensor.matmul(pl48[:], ones_sb[:], lpart[:])
        lg = pg.tile([1, 48], F32)
        nc.vector.tensor_add(lg[:], pl48[:], c2it_sb[:])
        # softmax over e' in groups of 4 (layout (p, b, e'))
        mx = pg.tile([1, 12], F32)
        nc.vector.tensor_reduce(mx[:], lg[:].rearrange('o (g e) -> o g e', e=4),
                                axis=AX.X, op=OP.max)
        for e in range(4):
            nc.vector.tensor_sub(lg[:].rearrange('o (g e) -> o g e', e=4)[:, :, e],
                                 lg[:].rearrange('o (g e) -> o g e', e=4)[:, :, e],
                                 mx[:])
        nc.scalar.activation(out=lg[:], in_=lg[:], func=AF.Exp)
        sm = pg.tile([1, 12], F32)
        nc.vector.tensor_reduce(sm[:], lg[:].rearrange('o (g e) -> o g e', e=4),
                                axis=AX.X, op=OP.add)
        nc.vector.reciprocal(sm[:], sm[:])
        for e in range(4):
            nc.vector.tensor_mul(gates_sb[:].rearrange('o (g e) -> o g e', e=4)[:, :, e],
                                 lg[:].rearrange('o (g e) -> o g e', e=4)[:, :, e],
                                 sm[:])
    with tc.tile_pool(name="dg" + sfx, bufs=1, space="DRAM") as dg:
        g_dram = dg.tile([48], F32)
        gdv = g_dram[:]
        dma(out=bass.AP(tensor=gdv.tensor, offset=gdv.offset,
                        ap=[[0, 1], [1, 48]]), in_=gates_sb[:])
        gd = g_dram[:]
        # gvqk rows: er 0:64 -> (p=0,e=er//16), er 64:128 -> (p=1,...); col b
        gq = gvqk_sb[:]
        gv3 = gvv_sb[:]
        for pi, dstap, dstoff in [(0, gq, 0), (1, gq, 256), (2, gv3, 0)]:
            for e in range(4):
                nc.gpsimd.dma_start(
                    out=bass.AP(tensor=dstap.tensor,
                                offset=dstap.offset + dstoff + e * 64,
                                ap=[[4, 16], [1, 4]]),
                    in_=bass.AP(tensor=gd.tensor, offset=gd.offset + pi * 16 + e,
                                ap=[[0, 16], [4, 4]]))

    if DEBUG:
        dma(out=g['dbg_stats'][:], in_=stats_ar[:])
        gdv2 = g['dbg_gates'][:]
        dma(out=bass.AP(tensor=gdv2.tensor, offset=gdv2.offset, ap=[[0, 1], [1, 48]]),
            in_=gates_sb[:])
    if PHASES < 4:
        return
    # ---------------- phase B ----------------
    with tc.tile_pool(name="pb" + sfx, bufs=2) as pb, \
         tc.tile_pool(name="ppx" + sfx, bufs=2, space="PSUM") as ppx, \
         tc.tile_pool(name="ppt" + sfx, bufs=2, space="PSUM") as ppt:
        for st in range(NST):
            for b in range(B):
                t = st * B + b
                hgqk = pb.tile([128, 128], F16, tag="hgqk")
                hgv = pb.tile([64, 128], F16, tag="hgv")
                nc.vector.tensor_scalar_mul(hgqk[:], hqk_all[:, t, :],
                                            gvqk_sb[:, b:b + 1])
                nc.vector.tensor_scalar_mul(hgv[:], hv_all[:, t, :],
                                            gvv_sb[:, b:b + 1])
                px = ppx.tile([128, 256], F32, tag="px")
                nc.tensor.matmul(px[:], hgqk[:], bbqk_sb[:], start=True, stop=False)
                nc.tensor.matmul(px[:], hgv[:], bbv_sb[:], start=False, stop=True)
                xf = pb.tile([128, 256], F16, tag="xf")
                nc.vector.tensor_scalar_mul(xf[:], base_all[:, st, b, :], 2.0)
                nc.vector.tensor_add(xf[:], xf[:], px[:])
                # v (no rope) + ones col
                nc.vector.tensor_copy(vaug_all[:, st, b, 0:64], xf[:, 192:256])
                # rope q (cols 0:128, 2 heads) and k (cols 128:192)
                xr = pb.tile([128, 192], F16, tag="xr")
                tmp1 = pb.tile([128, 64], F32, tag="tmp1")
                tmp2 = pb.tile([128, 64], F32, tag="tmp2")
                for (o0, nh) in [(0, 2), (128, 1)]:
                    ne = nh * 32
                    xfo = xf[:]
                    pstep = xfo.ap[0][0]
                    xe = bass.AP(tensor=xf.tensor, offset=xfo.offset + o0,
                                 ap=[[pstep, 128], [64, nh], [2, 32]])
                    xo = bass.AP(tensor=xf.tensor, offset=xfo.offset + o0 + 1,
                                 ap=[[pstep, 128], [64, nh], [2, 32]])
                    xro = xr[:]
                    prstep = xro.ap[0][0]
                    xre = bass.AP(tensor=xr.tensor, offset=xro.offset + o0,
                                  ap=[[prstep, 128], [64, nh], [2, 32]])
                    xroo = bass.AP(tensor=xr.tensor, offset=xro.offset + o0 + 1,
                                   ap=[[prstep, 128], [64, nh], [2, 32]])
                    cb = cos_sb[:, st, 0:ne].rearrange('p (h i) -> p h i', h=nh)
                    sb_ = sin_sb[:, st, 0:ne].rearrange('p (h i) -> p h i', h=nh)
                    T1 = tmp1[:, 0:ne].rearrange('p (h i) -> p h i', h=nh)
                    T2 = tmp2[:, 0:ne].rearrange('p (h i) -> p h i', h=nh)
                    nc.vector.tensor_mul(T1, xe, cb)
                    nc.vector.tensor_mul(T2, xo, sb_)
                    nc.vector.tensor_sub(xre, T1, T2)
                    nc.vector.tensor_mul(T1, xe, sb_)
                    nc.vector.tensor_mul(T2, xo, cb)
                    nc.vector.tensor_add(xroo, T1, T2)
                if DEBUG:
                    nc.gpsimd.dma_start(out=g['dbg_xr'][:, t, :], in_=xr[:])
                pt = ppt.tile([64, 384], F16, tag="pt")
                nc.tensor.transpose(pt[:, 0:128], xr[:, 0:64], ident_sb[:])
                nc.tensor.transpose(pt[:, 128:256], xr[:, 64:128], ident_sb[:])
                nc.tensor.transpose(pt[:, 256:384], xr[:, 128:192], ident_sb[:])
                nc.scalar.mul(qT_all[:, st, b, 0, :], pt[:, 0:128], 0.125)
                nc.scalar.mul(qT_all[:, st, b, 1, :], pt[:, 128:256], 0.125)
                nc.scalar.copy(kT_all[:, st, b, :], pt[:, 256:384])

    if PHASES < 5:
        return
    # ---------------- attention ----------------
    with tc.tile_pool(name="pat" + sfx, bufs=2) as pat, \
         tc.tile_pool(name="pps" + sfx, bufs=2, space="PSUM") as pps, \
         tc.tile_pool(name="ppa" + sfx, bufs=2, space="PSUM") as ppa:
        for b in range(B):
            for hh in range(2):
                probs = pat.tile([128, NST, 1024], BF16, tag="probs")
                for i in range(NST):
                    ki = kT_all[:, i, b, :]
                    for k5 in range(2):
                        c0 = max(512 * k5, i * 128)
                        c1v = 512 * (k5 + 1)
                        if c0 >= c1v:
                            continue
                        ln = c1v - c0
                        ps = pps.tile([128, 512], F32, tag="ps")
                        st0, p0 = divmod(c0, 128)
                        nst = ln // 128
                        rhs = qT_all[:, st0:st0 + nst, b, hh, :]
                        nc.tensor.matmul(ps[:, 0:ln], ki, rhs)
                        if c0 == i * 128:
                            nc.vector.tensor_add(ps[:, 0:128], ps[:, 0:128],
                                                 maskt_sb[:])
                        nc.scalar.activation(out=probs[:, i, c0:c0 + ln],
                                             in_=ps[:, 0:ln], func=AF.Exp)
                for j in range(NST):
                    pa_ = ppa.tile([128, 65], F32, tag="pa")
                    for i in range(j + 1):
                        nc.tensor.matmul(pa_[:], probs[:, i, 128 * j:128 * j + 128],
                                         vaug_all[:, i, b, :],
                                         start=(i == 0), stop=(i == j))
                    rc = pat.tile([128, 1], F32, tag="rc")
                    nc.vector.reciprocal(rc[:], pa_[:, 64:65])
                    nc.vector.tensor_scalar_mul(
                        att_all[:, j * B + b, 64 * hh:64 * hh + 64],
                        pa_[:, 0:64], rc[:])

    if DEBUG:
        nc.gpsimd.dma_start(out=g['dbg_att'][:], in_=att_all[:])
    # ---------------- output projection ----------------
    with tc.tile_pool(name="pw" + sfx, bufs=2) as pw, \
         tc.tile_pool(name="ppo" + sfx, bufs=2, space="PSUM") as ppo, \
         tc.tile_pool(name="ppat" + sfx, bufs=2, space="PSUM") as ppat:
        ov = g['outp'][:].rearrange('(b st p) d -> p st b d', p=128, st=NST)
        for st in range(NST):
            for b in range(B):
                t = st * B + b
                pT = ppat.tile([128, 128], F16, tag="pT")
                nc.tensor.transpose(pT[:], att_all[:, t, :], ident_sb[:])
                aT = pw.tile([128, 128], F16, tag="aT")
                nc.scalar.copy(aT[:], pT[:])
                po = ppo.tile([128, 1024], F32, tag="po")
                nc.tensor.matmul(po[:, 0:512], aT[:], wo_sb[:, 0:512])
                nc.tensor.matmul(po[:, 512:1024], aT[:], wo_sb[:, 512:1024])
                ob = pw.tile([128, 1024], F32, tag="ob")
                nc.scalar.copy(ob[:, 0:512], po[:, 0:512])
                nc.vector.tensor_copy(ob[:, 512:1024], po[:, 512:1024])
                dma(out=ov[:, st, b, :], in_=ob[:])


def build_kernel(repeat=1):
    key = (repeat, PHASES, DEBUG, APARTS)
    if key in _CACHE:
        return _CACHE[key]
    nc = bacc.Bacc()
    io = {}
    def din(name, shape, dt):
        io[name] = nc.dram_tensor(name, list(shape), dt, kind="ExternalInput")
    din('xt16', (8, 128, BS), F16)
    din('wqkv16', (8, 128, 256), F16)
    din('acat16', (8, 128, 192), F16)
    din('blq16', (64, 512), F16)
    din('blk16', (64, 256), F16)
    din('blv16', (64, 256), F16)
    din('bbqk16', (128, 256), F16)
    din('bbv16', (64, 256), F16)
    din('wg16', (S, 4, 1024), F16)
    din('c1_12', (S, 12), F32)
    din('c2it48', (48,), F32)
    din('itau3', (3,), F32)
    din('cosr', (S, 64), F32)
    din('sinr', (S, 64), F32)
    din('maskt', (128, 128), F32)
    din('wo16', (128, 1024), F16)
    io['outp'] = nc.dram_tensor('outp', [BS, 1024], F32, kind="ExternalOutput")
    if DEBUG:
        io['dbg_stats'] = nc.dram_tensor('dbg_stats', [128, NST, B, 18], F32, kind="ExternalOutput")
        io['dbg_gates'] = nc.dram_tensor('dbg_gates', [48], F32, kind="ExternalOutput")
        io['dbg_att'] = nc.dram_tensor('dbg_att', [128, NT, 128], F32, kind="ExternalOutput")
        io['dbg_xr'] = nc.dram_tensor('dbg_xr', [128, NT, 192], F32, kind="ExternalOutput")
    nc._kernel_io = io
    from contextlib import ExitStack
    with tile.TileContext(nc) as tc:
        for rep in range(repeat):
            with ExitStack() as ctx:
                _emit(nc, tc, ctx, rep)
    nc.finalize()
    _CACHE[key] = nc
    return nc


def prep_inputs(inputs):
    """Host-side sharding prep: returns in_maps (list of 8 dicts)."""
    f = np.float32
    x = np.asarray(inputs['x'], f)
    xT = np.ascontiguousarray(x.transpose(2, 0, 1).reshape(D, BS)).astype(np.float16)
    xt16 = np.ascontiguousarray(xT.reshape(8, 128, BS))
    cosr = np.ascontiguousarray(np.tile(np.asarray(inputs['cos'], f), (1, 2)))
    sinr = np.ascontiguousarray(np.tile(np.asarray(inputs['sin'], f), (1, 2)))
    maskt = np.ascontiguousarray(np.asarray(inputs['mask'], f)[0:128, 0:128].T)
    in_maps = []
    pr = {}
    for p, Of in [('q', H * HD), ('k', KVH * HD), ('v', KVH * HD)]:
        A = np.asarray(inputs[f'A_{p}'], f)
        Bm = np.asarray(inputs[f'B_{p}'], f)
        gg = np.asarray(inputs[f'g_{p}'], f)
        bb = np.asarray(inputs[f'b_{p}'], f)
        We = np.asarray(inputs[f'We_{p}'], f)
        tau = float(np.asarray(inputs[f'tau_{p}']))
        itau = 1.0 / max(tau, 1e-6)
        Acat = np.ascontiguousarray(A.transpose(1, 0, 2).reshape(D, NE * R))
        Bp = SCALING * Bm                      # [E,R,Of]
        gv = gg.reshape(NE, Of)
        Wgf = We.reshape(S, NE, Of, NE) * gv[None, :, :, None]
        C1 = Wgf.sum(axis=(1, 2)) * itau       # [S,4]
        C2 = (We.reshape(S, NE * Of, NE) * bb[None, :, None]).sum((0, 1)) * itau
        pr[p] = dict(Acat=Acat, Bp=Bp, Wgf=Wgf, C1=C1, C2=C2, itau=itau)
    c1_12 = np.concatenate([pr[p]['C1'] for p in 'qkv'], 1).astype(f)
    c2it48 = np.zeros(48, f)
    for pi, p in enumerate('qkv'):
        for b in range(B):
            c2it48[pi * 16 + b * 4:pi * 16 + b * 4 + 4] = pr[p]['C2']
    itau3 = np.array([pr[p]['itau'] for p in 'qkv'], f)
    acat16 = np.concatenate([pr[p]['Acat'] for p in 'qkv'], 1)  # [D,192]
    acat16 = np.ascontiguousarray(acat16.reshape(8, 128, 192)).astype(np.float16)

    wq = np.asarray(inputs['wq'], f)
    wk = np.asarray(inputs['wk'], f)
    wv = np.asarray(inputs['wv'], f)
    wo = np.asarray(inputs['wo'], f)
    for c in range(NC):
        qs = slice(128 * c, 128 * c + 128)
        ks = slice(64 * c, 64 * c + 64)
        wqkv = np.concatenate([wq[:, qs], wk[:, ks], wv[:, ks]], 1)   # [D,256]
        wqkv16 = np.ascontiguousarray(wqkv.reshape(8, 128, 256)).astype(np.float16)
        m = dict(xt16=xt16, wqkv16=wqkv16, acat16=acat16, c1_12=c1_12,
                 c2it48=c2it48, itau3=itau3, cosr=cosr, sinr=sinr, maskt=maskt)
        # lora block matrices
        blq = np.zeros((64, 512), f)
        blk = np.zeros((64, 256), f)
        blv = np.zeros((64, 256), f)
        for e in range(NE):
            blq[e * 16:e * 16 + 16, e * 128:e * 128 + 128] = pr['q']['Bp'][e][:, qs]
            blk[e * 16:e * 16 + 16, e * 64:e * 64 + 64] = pr['k']['Bp'][e][:, ks]
            blv[e * 16:e * 16 + 16, e * 64:e * 64 + 64] = pr['v']['Bp'][e][:, ks]
        bbqk = np.zeros((128, 256), f)
        bbv = np.zeros((64, 256), f)
        bbqk[0:64, 0:128] = pr['q']['Bp'][:, :, qs].reshape(64, 128)
        bbqk[64:128, 128:192] = pr['k']['Bp'][:, :, ks].reshape(64, 64)
        bbv[:, 192:256] = pr['v']['Bp'][:, :, ks].reshape(64, 64)
        m['blq16'] = blq.astype(np.float16)
        m['blk16'] = blk.astype(np.float16)
        m['blv16'] = blv.astype(np.float16)
        m['bbqk16'] = bbqk.astype(np.float16)
        m['bbv16'] = bbv.astype(np.float16)
        # Wg packed [S, 4e', 1024]: q 0:512, k 512:768, v 768:1024
        wg = np.zeros((S, 4, 1024), f)
        wg[:, :, 0:512] = pr['q']['Wgf'][:, :, qs, :].transpose(0, 3, 1, 2).reshape(S, 4, 512)
        wg[:, :, 512:768] = pr['k']['Wgf'][:, :, ks, :].transpose(0, 3, 1, 2).reshape(S, 4, 256)
        wg[:, :, 768:1024] = pr['v']['Wgf'][:, :, ks, :].transpose(0, 3, 1, 2).reshape(S, 4, 256)
        m['wg16'] = wg.astype(np.float16)
        m['wo16'] = wo[qs, :].astype(np.float16)
        in_maps.append(m)
    return in_maps


def run_on_device(in_maps, repeat=1):
    from concourse.bass_utils import run_bass_kernel_spmd
    nc = build_kernel(repeat)
    res = run_bass_kernel_spmd(nc, in_maps, list(range(NC)))
    return res


def _run_sim(in_maps):
    from concourse.bass_interp import MultiCoreSim
    nc = build_kernel(1)
    sim = MultiCoreSim(nc, NC, num_workers=NC)
    for c in range(NC):
        for name, arr in in_maps[c].items():
            sim.cores[c].tensor(name)[:] = arr
    sim.simulate()
    return [{'outp': np.asarray(sim.cores[c].tensor('outp'))} for c in range(NC)]


def kernel(**inputs):
    in_maps = prep_inputs(inputs)
    try:
        results = run_on_device(in_maps, repeat=1).results
    except Exception as e:
        sys.stderr.write(f"device run failed ({e}); falling back to CoreSim\n")
        results = _run_sim(in_maps)
    out = np.zeros((BS, 1024), np.float32)
    for c in range(NC):
        out += results[c]['outp']
    return out.reshape(B, S, 1024)



# revision 74
# speedup vs baseline: 16100.1716x; 16100.1716x over previous
# Trainium2 Bass kernel for nn_AttentionWithMoLE — 8-core tensor-parallel over heads.
#
# Sharding: core c owns q-heads {2c, 2c+1} (wq cols 128c:128c+128), kv-head c
# (wk/wv cols 64c:64c+64), wo rows 128c:128c+128. Host sums the 8 partial outputs.
#
# MoLE algebra used on device (validated against the reference in numpy):
#   xq = 2*base + sum_e gate_e * lora_e          (softmax gates sum to 1)
#   logits[b,e'] = sum_s r[b,s]*(P1[b,s,e'] - mu[b,s]*C1[s,e']) + C2[e']  (then /tau)
#   P1[b,s,e'] = sum_o base[o]*Wgb[s,o,e'] + sum_er h[er]*W2[s,er,e']
#     where Wgb = sum_e Wgf[s,e,o,e'] (feature-shard) and W2 = Bp-folded Wgf
#     (replicated, /8).  s1 = 4*sum_o base + h.b2 rides as a 5th "e'" column.
#   s2 = sum_f ecat^2 (feature shard), ecat = pl + base broadcast over e.
# Stats ([p,st,b,18]: per proj P1[4]+s1, then s2 x3) AllReduce'd, gates
# computed replicated, then pass B builds q/k/v + RoPE + causal attention
# (scoresT orientation, exp without max-subtraction, bf16 probs, fused rowsum
# via a ones-column appended to V) + output projection (fp16 partials).
import sys
import numpy as np

sys.path.insert(0, '/opt/trn_rl_repo')

import concourse.bass as bass
import concourse.bacc as bacc
import concourse.tile as tile
import concourse.mybir as mybir
from concourse.masks import make_identity

NC = 8
B, S, D, H, KVH, HD, NE, R = 4, 1024, 1024, 16, 8, 64, 4, 16
SCALING, GEPS = 2.0, 1e-6
BS = B * S
NST = S // 128           # 8 s-tiles
NT = NST * B             # 32 token tiles of 128
F16 = mybir.dt.float16
BF16 = mybir.dt.bfloat16
F32 = mybir.dt.float32
AX = mybir.AxisListType
OP = mybir.AluOpType
AF = mybir.ActivationFunctionType

_CACHE = {}
PHASES = 6
APARTS = 7
NOCOLL = False   # timing mode: collectives not supported inside For_i loops


def _bcast_ap(ap, ins):
    """Insert step-0 broadcast dims: ins = list of (pos, count) into ap.ap."""
    dims = [list(d) for d in ap.ap]
    for pos, count in ins:
        dims.insert(pos, [0, count])
    return bass.AP(tensor=ap.tensor, offset=ap.offset, ap=dims)


def _emit(nc, tc, ctx, rep):
    g = nc._kernel_io  # dict of dram handles
    sfx = f"_r{rep}"

    # ---------------- resident pools ----------------
    res = ctx.enter_context(tc.tile_pool(name="res" + sfx, bufs=1))
    pb16_all = res.tile([128, NT, 448], F16)    # [q128|Aq64|k64|Ak64|v64|Av64]
    hqk_all = res.tile([128, NT, 128], F16)     # [er(q0:64,k64:128), t, tok]
    hv_all = res.tile([64, NT, 128], F16)
    xfb_all = res.tile([128, NT, 256], F16)     # 2*base, [q128|k64|v64]
    qT_all = res.tile([64, NST, B, 2, 128], F16)  # [hd, st, b, head, tok] (prescaled 1/8)
    kT_all = res.tile([64, NST, B, 128], F16)
    vaug_all = res.tile([128, NST, B, 65], BF16)
    att_all = res.tile([128, NT, 128], F16)     # [tok(sq), t, o(2 heads)]
    stats_all = res.tile([128, NST, B, 18], F32)
    stats16 = res.tile([128, NST, B, 18], F16)
    stats_ar = res.tile([128, NST, B, 18], F16)
    wqkv_sb = res.tile([128, 8, 451], F16)      # [wq|Aq|wk|Ak|wv|Av|s1cols] per chunk
    blq_sb = res.tile([64, 512], F16)
    blk_sb = res.tile([128, 256], F16)
    blv_sb = res.tile([64, 256], F16)
    bbqk_sb = res.tile([128, 256], F16)
    bbv_sb = res.tile([64, 256], F16)
    wo_sb = res.tile([128, 1024], F16)
    cos_sb = res.tile([128, NST, 96], F32)
    sin_sb = res.tile([128, NST, 96], F32)
    c1_sb = res.tile([128, NST, 12], F32)
    c2it_sb = res.tile([1, 48], F32)
    itau_sb = res.tile([128, 3], F32)
    maskt_sb = res.tile([128, 128], F32)
    ident_sb = res.tile([128, 128], F16)
    ones_sb = res.tile([128, 1], F32)
    geps_sb = res.tile([128, 1], F32)
    zero_sb = res.tile([128, 1], F32)
    gvqk_sb = res.tile([128, B], F32)
    gvv_sb = res.tile([64, B], F32)
    gates_sb = res.tile([1, 48], F16)
    g48_sb = res.tile([48, 1], F16)
    g48x4_sb = res.tile([48, B], F16)
    selqk_sb = res.tile([48, 128], F16)
    selv_sb = res.tile([48, 64], F16)
    bmask_sb = res.tile([48, B], F16)

    dma = nc.sync.dma_start
    dma(out=wqkv_sb[:], in_=g['wqkv16'][:])
    dma(out=blq_sb[:], in_=g['blq16'][:])
    dma(out=blk_sb[64:128, :], in_=g['blk16'][:])
    dma(out=blv_sb[:], in_=g['blv16'][:])
    dma(out=bbqk_sb[:], in_=g['bbqk16'][:])
    dma(out=bbv_sb[:], in_=g['bbv16'][:])
    dma(out=wo_sb[:], in_=g['wo16'][:])
    dma(out=cos_sb[:], in_=g['cosr'][:])
    dma(out=sin_sb[:], in_=g['sinr'][:])
    dma(out=c1_sb[:], in_=g['c1_12'][:])
    c2 = g['c2it48'][:]
    dma(out=c2it_sb[:], in_=bass.AP(tensor=c2.tensor, offset=c2.offset,
                                    ap=[[0, 1], [1, 48]]))
    it = g['itau3'][:]
    dma(out=itau_sb[:], in_=bass.AP(tensor=it.tensor, offset=it.offset,
                                    ap=[[0, 128], [1, 3]]))
    dma(out=maskt_sb[:], in_=g['maskt'][:])
    dma(out=selqk_sb[:], in_=g['selqk'][:])
    dma(out=selv_sb[:], in_=g['selv'][:])
    dma(out=bmask_sb[:], in_=g['bmask'][:])
    make_identity(nc, ident_sb[:])
    nc.vector.memset(ones_sb[:], 1.0)
    nc.vector.memset(geps_sb[:], GEPS)
    nc.vector.memset(zero_sb[:], 0.0)
    nc.vector.memset(vaug_all[:, :, :, 64:65], 1.0)

    # ---------------- phase A ----------------
    if PHASES < 1:
        return
    segs = [(0, 192), (192, 320), (320, 448)]   # per-proj [base|h] blocks
    with tc.tile_pool(name="pa" + sfx, bufs=2) as pa, \
         tc.tile_pool(name="ppb" + sfx, bufs=2, space="PSUM") as ppb, \
         tc.tile_pool(name="ppt" + sfx, bufs=2, space="PSUM") as ppt, \
         tc.tile_pool(name="ppl" + sfx, bufs=2, space="PSUM") as ppl:
        for st in range(NST):
            wcat_t = pa.tile([128, 4, 448], F16, tag="wcat")
            dma(out=wcat_t[:], in_=g['wcat16'][:][st])
            xt = pa.tile([128, B, 8, 128], F16, tag="xt")
            dma(out=xt[:], in_=g['xt16'][:][st])
            for b in range(B):
                t = st * B + b
                pbase = ppb.tile([128, 451], F32, tag="pbase")
                for ch in range(8):
                    nc.tensor.matmul(pbase[:], xt[:, b, ch, :], wqkv_sb[:, ch, :],
                                     start=(ch == 0), stop=(ch == 7))
                pb16 = pb16_all[:, t, :]
                nc.scalar.copy(pb16, pbase[:, 0:448])
                sa = stats_all
                # s1 (exact, via matmul cols) -> stats cols 4, 9, 14
                pbv = pbase[:]
                s1dst = bass.AP(tensor=sa.tensor, offset=sa[:, st, b, 4:5].offset,
                                ap=[list(sa[:, st, b, 4:5].ap[0]), [5, 3]])
                nc.scalar.copy(s1dst, pbase[:, 448:451])
                # er-major h via transposes: hqk [q-h | k-h], hv
                pt = ppt.tile([128, 256], F16, tag="pt")
                nc.tensor.transpose(pt[0:64, 0:128], pb16_all[:, t, 128:192],
                                    ident_sb[:])
                nc.tensor.transpose(pt[64:128, 0:128], pb16_all[:, t, 256:320],
                                    ident_sb[:])
                nc.tensor.transpose(pt[0:64, 128:256], pb16_all[:, t, 384:448],
                                    ident_sb[:])
                nc.scalar.copy(hqk_all[:, t, :], pt[:, 0:128])
                nc.scalar.copy(hv_all[:, t, :], pt[0:64, 128:256])
                if APARTS < 2:
                    continue
                # lora_e via block-diag B' (K=64)
                pl = ppl.tile([128, 1024], F32, tag="pl")
                nc.tensor.matmul(pl[:, 0:512], hqk_all[0:64, t, :], blq_sb[:])
                nc.tensor.matmul(pl[:, 512:768], hqk_all[64:128, t, :], blk_sb[64:128, :])
                nc.tensor.matmul(pl[:, 768:1024], hv_all[:, t, :], blv_sb[:])
                if APARTS < 4:
                    continue
                # ecat = pl + base (broadcast over e); s2 = sum ecat^2 (fused)
                # (DVE reads pl from PSUM directly; Pool is not PSUM-capable)
                ecat = pa.tile([128, 1024], F16, tag="ecat")
                junk = pa.tile([128, 512], F16, tag="junk")
                nc.vector.tensor_add(
                    ecat[:, 0:512].rearrange('p (e o) -> p e o', e=4),
                    pl[:, 0:512].rearrange('p (e o) -> p e o', e=4),
                    _bcast_ap(pb16_all[:, t, 0:128], [(1, 4)]))
                nc.vector.tensor_add(
                    ecat[:, 512:768].rearrange('p (e o) -> p e o', e=4),
                    pl[:, 512:768].rearrange('p (e o) -> p e o', e=4),
                    _bcast_ap(pb16_all[:, t, 192:256], [(1, 4)]))
                nc.vector.tensor_add(
                    ecat[:, 768:1024].rearrange('p (e o) -> p e o', e=4),
                    pl[:, 768:1024].rearrange('p (e o) -> p e o', e=4),
                    _bcast_ap(pb16_all[:, t, 320:384], [(1, 4)]))
                for ci, (f0, f1) in enumerate([(0, 512), (512, 768), (768, 1024)]):
                    nc.scalar.activation(
                        out=junk[:, 0:f1 - f0], in_=ecat[:, f0:f1],
                        func=AF.Square,
                        accum_out=sa[:, st, b, 15 + ci:16 + ci])
                # P1: broadcast mul (Pool) + 3 segmented reduces (DVE)
                wm = pa.tile([128, 4, 448], F16, tag="wm")
                nc.gpsimd.tensor_mul(wm[:], wcat_t[:],
                                     _bcast_ap(pb16_all[:, t, :], [(1, 4)]))
                nc.vector.tensor_reduce(sa[:, st, b, 0:4], wm[:, :, 0:192],
                                        axis=AX.X, op=OP.add)
                nc.vector.tensor_reduce(sa[:, st, b, 5:9], wm[:, :, 192:320],
                                        axis=AX.X, op=OP.add)
                nc.vector.tensor_reduce(sa[:, st, b, 10:14], wm[:, :, 320:448],
                                        axis=AX.X, op=OP.add)

    if PHASES < 2:
        return
    # gate-independent 2*base precompute (overlaps the AllReduce below)
    for t in range(NT):
        nc.vector.tensor_scalar_mul(xfb_all[:, t, 0:128],
                                    pb16_all[:, t, 0:128], 2.0)
        pbv = pb16_all[:, t, 0:1]
        srckv = bass.AP(tensor=pb16_all.tensor, offset=pbv.offset + 192,
                        ap=[list(pbv.ap[0]), [128, 2], [1, 64]])
        nc.vector.tensor_scalar_mul(
            xfb_all[:, t, 128:256].rearrange('p (s o) -> p s o', s=2), srckv, 2.0)
    # ---------------- AllReduce ----------------
    # fp16 collective payload (144KB): stats precision is ample in fp16
    nc.scalar.copy(stats16[:], stats_all[:])
    with tc.tile_pool(name="dr" + sfx, bufs=1, space="DRAM") as dr:
        ar_in = dr.tile([128, NST, B, 18], F16)
        ar_out = dr.tile([128, NST, B, 18], F16)
        dma(out=ar_in[:], in_=stats16[:])
        if NOCOLL:
            dma(out=ar_out[:], in_=ar_in[:])
        else:
            nc.gpsimd.collective_compute(
                "AllReduce", OP.add, replica_groups=[list(range(NC))],
                ins=[ar_in[:]], outs=[ar_out[:]])
        dma(out=stats_ar[:], in_=ar_out[:])

    if PHASES < 3:
        return
    # ---------------- gates (replicated) ----------------
    with tc.tile_pool(name="pg" + sfx, bufs=1) as pg, \
         tc.tile_pool(name="pgp" + sfx, bufs=1, space="PSUM") as pgp:
        lpart = pg.tile([128, 48], F32)
        for pi, Ff in enumerate([NE * H * HD, NE * KVH * HD, NE * KVH * HD]):
            s1 = stats_ar[:, :, :, 5 * pi + 4]
            s2 = stats_ar[:, :, :, 15 + pi]
            mu = pg.tile([128, NST, B], F32, tag=f"mu{pi}")
            vr = pg.tile([128, NST, B], F32, tag=f"vr{pi}")
            rr = pg.tile([128, NST, B], F32, tag=f"rr{pi}")
            rm = pg.tile([128, NST, B], F32, tag=f"rm{pi}")
            t1 = pg.tile([128, NST, B], F32, tag=f"t1{pi}")
            t2 = pg.tile([128, NST, B], F32, tag=f"t2{pi}")
            nc.vector.tensor_scalar_mul(mu[:], s1, 1.0 / Ff)
            nc.vector.tensor_scalar_mul(vr[:], s2, 1.0 / Ff)
            nc.vector.tensor_mul(t1[:], mu[:], mu[:])
            nc.vector.tensor_sub(vr[:], vr[:], t1[:])
            nc.scalar.activation(out=rr[:], in_=vr[:], func=AF.Sqrt, bias=geps_sb[:],
                                 scale=1.0)
            nc.vector.reciprocal(rr[:], rr[:])
            nc.vector.tensor_scalar_mul(rr[:], rr[:], itau_sb[:, pi:pi + 1])
            nc.vector.tensor_mul(rm[:], rr[:], mu[:])
            for e in range(4):
                p1 = stats_ar[:, :, :, 5 * pi + e]
                c1col = c1_sb[:, :, 4 * pi + e]
                nc.vector.tensor_mul(t1[:], p1, rr[:])
                nc.vector.tensor_mul(
                    t2[:].rearrange('p st b -> p st b'), _bcast_ap(c1col, [(2, B)]),
                    rm[:])
                nc.vector.tensor_sub(t1[:], t1[:], t2[:])
                # reduce over st keeping b -> columns p*16 + b*4 + e
                src = bass.AP(tensor=t1.tensor, offset=t1[:].offset,
                              ap=[list(t1[:].ap[0]), [1, B], [B, NST]])
                dst = bass.AP(tensor=lpart.tensor,
                              offset=lpart[:].offset + 16 * pi + e,
                              ap=[list(lpart[:].ap[0]), [4, B]])
                nc.vector.tensor_reduce(dst, src, axis=AX.X, op=OP.add)
        pl48 = pgp.tile([1, 48], F32)
        nc.tensor.matmul(pl48[:], ones_sb[:], lpart[:])
        lg = pg.tile([1, 48], F32)
        nc.vector.tensor_add(lg[:], pl48[:], c2it_sb[:])
        # softmax over e' in groups of 4 (layout (p, b, e'))
        mx = pg.tile([1, 12], F32)
        nc.vector.tensor_reduce(mx[:], lg[:].rearrange('o (g e) -> o g e', e=4),
                                axis=AX.X, op=OP.max)
        for e in range(4):
            nc.vector.tensor_sub(lg[:].rearrange('o (g e) -> o g e', e=4)[:, :, e],
                                 lg[:].rearrange('o (g e) -> o g e', e=4)[:, :, e],
                                 mx[:])
        nc.scalar.activation(out=lg[:], in_=lg[:], func=AF.Exp)
        sm = pg.tile([1, 12], F32)
        nc.vector.tensor_reduce(sm[:], lg[:].rearrange('o (g e) -> o g e', e=4),
                                axis=AX.X, op=OP.add)
        nc.vector.reciprocal(sm[:], sm[:])
        for e in range(4):
            nc.vector.tensor_mul(gates_sb[:].rearrange('o (g e) -> o g e', e=4)[:, :, e],
                                 lg[:].rearrange('o (g e) -> o g e', e=4)[:, :, e],
                                 sm[:])
        # broadcast gates to per-er rows: transpose to 48 partitions, expand
        # per-b columns via mask, then one-hot selection matmuls.
        g48T = pgp.tile([48, 1], F16)
        nc.tensor.transpose(g48T[:], gates_sb[:], ident_sb[0:1, 0:1])
        nc.scalar.copy(g48_sb[:], g48T[:])
        nc.vector.tensor_mul(g48x4_sb[:], bmask_sb[:],
                             _bcast_ap(g48_sb[:, 0], [(1, B)]))
        gvp = pgp.tile([128, B], F32)
        gvvp = pgp.tile([64, B], F32)
        nc.tensor.matmul(gvp[:], selqk_sb[:], g48x4_sb[:])
        nc.tensor.matmul(gvvp[:], selv_sb[:], g48x4_sb[:])
        nc.scalar.copy(gvqk_sb[:], gvp[:])
        nc.scalar.copy(gvv_sb[:], gvvp[:])

    if PHASES < 4:
        return
    # ---------------- phase B ----------------
    with tc.tile_pool(name="pb" + sfx, bufs=2) as pb, \
         tc.tile_pool(name="ppx" + sfx, bufs=2, space="PSUM") as ppx, \
         tc.tile_pool(name="ppt2" + sfx, bufs=2, space="PSUM") as ppt2:
        for b in range(B):
            for st in range(NST):
                t = st * B + b
                hgqk = pb.tile([128, 128], F16, tag="hgqk")
                hgv = pb.tile([64, 128], F16, tag="hgv")
                nc.gpsimd.tensor_scalar_mul(hgqk[:], hqk_all[:, t, :],
                                            gvqk_sb[:, b:b + 1])
                nc.gpsimd.tensor_scalar_mul(hgv[:], hv_all[:, t, :],
                                            gvv_sb[:, b:b + 1])
                px = ppx.tile([128, 256], F32, tag="px")
                nc.tensor.matmul(px[:], hgqk[:], bbqk_sb[:], start=True, stop=False)
                nc.tensor.matmul(px[:], hgv[:], bbv_sb[:], start=False, stop=True)
                xf = pb.tile([128, 192], F16, tag="xf")
                # v goes straight to vaug (no rope); q/k columns into xf
                nc.vector.tensor_add(xf[:], xfb_all[:, t, 0:192], px[:, 0:192])
                nc.vector.tensor_add(vaug_all[:, st, b, 0:64],
                                     xfb_all[:, t, 192:256], px[:, 192:256])
                # rope q (cols 0:128, 2 heads) and k (128:192) in one 3-head pass
                xr = pb.tile([128, 192], F16, tag="xr")
                tmp1 = pb.tile([128, 96], F32, tag="tmp1")
                tmp2 = pb.tile([128, 96], F32, tag="tmp2")
                tmp3 = pb.tile([128, 96], F32, tag="tmp3")
                tmp4 = pb.tile([128, 96], F32, tag="tmp4")
                xfo = xf[:]
                pstep = xfo.ap[0][0]
                xe = bass.AP(tensor=xf.tensor, offset=xfo.offset,
                             ap=[[pstep, 128], [64, 3], [2, 32]])
                xo = bass.AP(tensor=xf.tensor, offset=xfo.offset + 1,
                             ap=[[pstep, 128], [64, 3], [2, 32]])
                xro = xr[:]
                prstep = xro.ap[0][0]
                xre = bass.AP(tensor=xr.tensor, offset=xro.offset,
                              ap=[[prstep, 128], [64, 3], [2, 32]])
                xroo = bass.AP(tensor=xr.tensor, offset=xro.offset + 1,
                               ap=[[prstep, 128], [64, 3], [2, 32]])
                cb = cos_sb[:, st, :].rearrange('p (h i) -> p h i', h=3)
                sb_ = sin_sb[:, st, :].rearrange('p (h i) -> p h i', h=3)
                T1 = tmp1[:].rearrange('p (h i) -> p h i', h=3)
                T2 = tmp2[:].rearrange('p (h i) -> p h i', h=3)
                T3 = tmp3[:].rearrange('p (h i) -> p h i', h=3)
                T4 = tmp4[:].rearrange('p (h i) -> p h i', h=3)
                nc.vector.tensor_mul(T1, xe, cb)
                nc.vector.tensor_mul(T2, xo, sb_)
                nc.vector.tensor_sub(xre, T1, T2)
                nc.gpsimd.tensor_mul(T3, xe, sb_)
                nc.gpsimd.tensor_mul(T4, xo, cb)
                nc.gpsimd.tensor_add(xroo, T3, T4)
                pt = ppt2.tile([64, 384], F16, tag="pt")
                nc.tensor.transpose(pt[:, 0:128], xr[:, 0:64], ident_sb[:])
                nc.tensor.transpose(pt[:, 128:256], xr[:, 64:128], ident_sb[:])
                nc.tensor.transpose(pt[:, 256:384], xr[:, 128:192], ident_sb[:])
                nc.scalar.mul(qT_all[:, st, b, :, :], pt[:, 0:256], 0.125)
                nc.scalar.copy(kT_all[:, st, b, :], pt[:, 256:384])

    if PHASES < 5:
        return
    # ---------------- attention ----------------
    with tc.tile_pool(name="pat" + sfx, bufs=2) as pat, \
         tc.tile_pool(name="pps" + sfx, bufs=2, space="PSUM") as pps, \
         tc.tile_pool(name="ppa" + sfx, bufs=2, space="PSUM") as ppa:
        for b in range(B):
            for hh in range(2):
                probs = pat.tile([128, NST, 1024], BF16, tag="probs")
                for i in range(NST):
                    ki = kT_all[:, i, b, :]
                    c0 = i * 128
                    ps = pps.tile([128, 1024], F32, tag="ps")
                    for (m0, m1) in [(c0, 512), (max(512, c0), 1024)]:
                        if m0 >= m1:
                            continue
                        st0 = m0 // 128
                        rhs = qT_all[:, st0:(m1 // 128), b, hh, :]
                        nc.tensor.matmul(ps[:, m0:m1], ki, rhs)
                    nc.vector.tensor_add(ps[:, c0:c0 + 128], ps[:, c0:c0 + 128],
                                         maskt_sb[:])
                    nc.scalar.activation(out=probs[:, i, c0:1024],
                                         in_=ps[:, c0:1024], func=AF.Exp)
                for j in range(NST):
                    pa_ = ppa.tile([128, 65], F32, tag="pa")
                    for i in range(j + 1):
                        nc.tensor.matmul(pa_[:], probs[:, i, 128 * j:128 * j + 128],
                                         vaug_all[:, i, b, :],
                                         start=(i == 0), stop=(i == j))
                    rc = pat.tile([128, 1], F32, tag="rc")
                    nc.vector.reciprocal(rc[:], pa_[:, 64:65])
                    nc.vector.tensor_scalar_mul(
                        att_all[:, j * B + b, 64 * hh:64 * hh + 64],
                        pa_[:, 0:64], rc[:])

    if PHASES < 6:
        return
    # ---------------- output projection ----------------
    with tc.tile_pool(name="pw" + sfx, bufs=2) as pw, \
         tc.tile_pool(name="ppo" + sfx, bufs=2, space="PSUM") as ppo, \
         tc.tile_pool(name="ppat" + sfx, bufs=2, space="PSUM") as ppat:
        op_ap = g['outp'][:]
        for b in range(B):
            ob = pw.tile([128, NST, 1024], F16, tag="ob")
            for st in range(NST):
                t = st * B + b
                pT = ppat.tile([128, 128], F16, tag="pT")
                nc.tensor.transpose(pT[:], att_all[:, t, :], ident_sb[:])
                aT = pw.tile([128, 128], F16, tag="aT")
                nc.scalar.copy(aT[:], pT[:])
                po = ppo.tile([128, 1024], F32, tag="po")
                nc.tensor.matmul(po[:, 0:512], aT[:], wo_sb[:, 0:512])
                nc.tensor.matmul(po[:, 512:1024], aT[:], wo_sb[:, 512:1024])
                nc.scalar.copy(ob[:, st, 0:512], po[:, 0:512])
                nc.vector.tensor_copy(ob[:, st, 512:1024], po[:, 512:1024])
            # dram AP permuted (p-major, st, d) to match the SBUF tile order
            dma(out=bass.AP(tensor=op_ap.tensor, offset=op_ap.offset + b * 1024,
                            ap=[[B * 1024, 128], [128 * B * 1024, NST], [1, 1024]]),
                in_=ob[:])


def build_kernel(repeat=1, loopn=0):
    key = (repeat, PHASES, APARTS, loopn, NOCOLL)
    if key in _CACHE:
        return _CACHE[key]
    nc = bacc.Bacc()
    io = {}
    def din(name, shape, dt):
        io[name] = nc.dram_tensor(name, list(shape), dt, kind="ExternalInput")
    din('xt16', (NST, 128, B, 8, 128), F16)
    din('wqkv16', (128, 8, 451), F16)
    din('wcat16', (NST, 128, 4, 448), F16)
    din('blq16', (64, 512), F16)
    din('blk16', (64, 256), F16)
    din('blv16', (64, 256), F16)
    din('bbqk16', (128, 256), F16)
    din('bbv16', (64, 256), F16)
    din('c1_12', (128, NST, 12), F32)
    din('c2it48', (48,), F32)
    din('itau3', (3,), F32)
    din('cosr', (128, NST, 96), F32)
    din('sinr', (128, NST, 96), F32)
    din('maskt', (128, 128), F32)
    din('selqk', (48, 128), F16)
    din('selv', (48, 64), F16)
    din('bmask', (48, B), F16)
    din('wo16', (128, 1024), F16)
    io['outp'] = nc.dram_tensor('outp', [NST, 128, B, 1024], F16,
                                kind="ExternalOutput")
    nc._kernel_io = io
    from contextlib import ExitStack
    with tile.TileContext(nc) as tc:
        if loopn:
            # hardware loop: same body executed loopn times (timing mode)
            with tc.For_i(0, loopn):
                with ExitStack() as ctx:
                    _emit(nc, tc, ctx, 0)
        else:
            for rep in range(repeat):
                with ExitStack() as ctx:
                    _emit(nc, tc, ctx, rep)
    nc.finalize()
    _CACHE[key] = nc
    return nc


def prep_inputs(inputs):
    """Host-side sharding prep: returns in_maps (list of 8 dicts)."""
    f = np.float32
    x = np.asarray(inputs['x'], f)
    # xt16[st, p, b, ch, tk] = x[b, st*128+tk, ch*128+p]
    xr8 = np.asarray(x.transpose(2, 0, 1), np.float16).reshape(8, 128, B, NST, 128)
    xt16 = np.ascontiguousarray(xr8.transpose(3, 1, 2, 0, 4))
    cos3 = np.tile(np.asarray(inputs['cos'], f), (1, 3)).reshape(NST, 128, 96)
    sin3 = np.tile(np.asarray(inputs['sin'], f), (1, 3)).reshape(NST, 128, 96)
    cosr = np.ascontiguousarray(cos3.transpose(1, 0, 2))
    sinr = np.ascontiguousarray(sin3.transpose(1, 0, 2))
    maskt = np.ascontiguousarray(np.asarray(inputs['mask'], f)[0:128, 0:128].T)
    # gate broadcast helpers: gates flat col = pi*16 + b*4 + e
    selqk = np.zeros((48, 128), np.float16)
    for er in range(128):
        pi, e = er // 64, (er % 64) // 16
        for b in range(B):
            selqk[pi * 16 + b * 4 + e, er] = 1.0
    selv = np.zeros((48, 64), np.float16)
    for er in range(64):
        for b in range(B):
            selv[2 * 16 + b * 4 + er // 16, er] = 1.0
    bmask = np.zeros((48, B), np.float16)
    for fl in range(48):
        bmask[fl, (fl % 16) // 4] = 1.0
    in_maps = []
    pr = {}
    for p, Of in [('q', H * HD), ('k', KVH * HD), ('v', KVH * HD)]:
        A = np.asarray(inputs[f'A_{p}'], f)
        Bm = np.asarray(inputs[f'B_{p}'], f)
        gg = np.asarray(inputs[f'g_{p}'], f)
        bb = np.asarray(inputs[f'b_{p}'], f)
        We = np.asarray(inputs[f'We_{p}'], f)
        tau = float(np.asarray(inputs[f'tau_{p}']))
        itau = 1.0 / max(tau, 1e-6)
        Acat = np.ascontiguousarray(A.transpose(1, 0, 2).reshape(D, NE * R))
        Bp = SCALING * Bm                      # [E,R,Of]
        gv = gg.reshape(NE, Of)
        Wgf = We.reshape(S, NE, Of, NE) * gv[None, :, :, None]
        Wgb = Wgf.sum(axis=1)                  # [S, Of, 4]
        W2 = np.einsum('ero,seoE->serE', Bp, Wgf).reshape(S, NE * R, NE) / NC
        b2 = Bp.sum(axis=2).reshape(NE * R) / NC
        C1 = Wgf.sum(axis=(1, 2)) * itau       # [S,4]
        C2 = (We.reshape(S, NE * Of, NE) * bb[None, :, None]).sum((0, 1)) * itau
        pr[p] = dict(Acat=Acat, Bp=Bp, Wgb=Wgb, W2=W2, b2=b2, C1=C1, C2=C2,
                     itau=itau)
    c1_12 = np.ascontiguousarray(
        np.concatenate([pr[p]['C1'] for p in 'qkv'], 1).astype(f)
        .reshape(NST, 128, 12).transpose(1, 0, 2))
    c2it48 = np.zeros(48, f)
    for pi, p in enumerate('qkv'):
        for b in range(B):
            c2it48[pi * 16 + b * 4:pi * 16 + b * 4 + 4] = pr[p]['C2']
    itau3 = np.array([pr[p]['itau'] for p in 'qkv'], f)

    wq = np.asarray(inputs['wq'], f)
    wk = np.asarray(inputs['wk'], f)
    wv = np.asarray(inputs['wv'], f)
    wo = np.asarray(inputs['wo'], f)
    # s1 columns (exact full-feature sums, /NC since every core computes them)
    s1cols = np.stack(
        [(4.0 * Wfull.sum(axis=1) / NC + pr[p]['Acat'] @ pr[p]['b2'])
         for p, Wfull in [('q', wq), ('k', wk), ('v', wv)]], axis=1)  # [D,3]
    for c in range(NC):
        qs = slice(128 * c, 128 * c + 128)
        ks = slice(64 * c, 64 * c + 64)
        # rhs chunks: [wq(128)|Aq(64)|wk(64)|Ak(64)|wv(64)|Av(64)|s1(3)] per ch
        wqkv = np.concatenate(
            [wq[:, qs], pr['q']['Acat'], wk[:, ks], pr['k']['Acat'],
             wv[:, ks], pr['v']['Acat'], s1cols], 1)             # [D,451]
        wqkv16 = np.ascontiguousarray(
            wqkv.reshape(8, 128, 451).transpose(1, 0, 2)).astype(np.float16)
        # wcat [S -> (st,p), 4, 448]
        wcat = np.zeros((S, 4, 448), f)
        for pi_, (p, sh, o0) in enumerate([('q', qs, 0), ('k', ks, 192),
                                           ('v', ks, 320)]):
            wcat[:, :, o0:o0 + (128 if p == 'q' else 64)] = \
                pr[p]['Wgb'][:, sh, :].transpose(0, 2, 1)
            h0 = o0 + (128 if p == 'q' else 64)
            wcat[:, :, h0:h0 + 64] = pr[p]['W2'].transpose(0, 2, 1)
        wcat16 = np.ascontiguousarray(
            wcat.reshape(NST, 128, 4, 448)).astype(np.float16)
        m = dict(xt16=xt16, wqkv16=wqkv16, wcat16=wcat16, c1_12=c1_12,
                 c2it48=c2it48, itau3=itau3, cosr=cosr, sinr=sinr, maskt=maskt,
                 selqk=selqk, selv=selv, bmask=bmask)
        # lora block matrices
        blq = np.zeros((64, 512), f)
        blk = np.zeros((64, 256), f)
        blv = np.zeros((64, 256), f)
        for e in range(NE):
            blq[e * 16:e * 16 + 16, e * 128:e * 128 + 128] = pr['q']['Bp'][e][:, qs]
            blk[e * 16:e * 16 + 16, e * 64:e * 64 + 64] = pr['k']['Bp'][e][:, ks]
            blv[e * 16:e * 16 + 16, e * 64:e * 64 + 64] = pr['v']['Bp'][e][:, ks]
        bbqk = np.zeros((128, 256), f)
        bbv = np.zeros((64, 256), f)
        bbqk[0:64, 0:128] = pr['q']['Bp'][:, :, qs].reshape(64, 128)
        bbqk[64:128, 128:192] = pr['k']['Bp'][:, :, ks].reshape(64, 64)
        bbv[:, 192:256] = pr['v']['Bp'][:, :, ks].reshape(64, 64)
        m['blq16'] = blq.astype(np.float16)
        m['blk16'] = blk.astype(np.float16)
        m['blv16'] = blv.astype(np.float16)
        m['bbqk16'] = bbqk.astype(np.float16)
        m['bbv16'] = bbv.astype(np.float16)
        m['wo16'] = wo[qs, :].astype(np.float16)
        in_maps.append(m)
    return in_maps


def run_on_device(in_maps, repeat=1, loopn=0):
    from concourse.bass_utils import run_bass_kernel_spmd
    nc = build_kernel(repeat, loopn)
    res = run_bass_kernel_spmd(nc, in_maps, list(range(NC)))
    return res


def _run_sim(in_maps):
    from concourse.bass_interp import MultiCoreSim
    nc = build_kernel(1)
    sim = MultiCoreSim(nc, NC, num_workers=NC)
    for c in range(NC):
        for name, arr in in_maps[c].items():
            sim.cores[c].tensor(name)[:] = arr
    sim.simulate()
    return [{'outp': np.asarray(sim.cores[c].tensor('outp'))} for c in range(NC)]


def kernel(**inputs):
    in_maps = prep_inputs(inputs)
    try:
        results = run_on_device(in_maps, repeat=1).results
    except Exception as e:
        sys.stderr.write(f"device run failed ({e}); falling back to CoreSim\n")
        results = _run_sim(in_maps)
    out = np.zeros((NST, 128, B, 1024), np.float32)
    for c in range(NC):
        out += np.asarray(results[c]['outp'], np.float32)
    return np.ascontiguousarray(out.transpose(2, 0, 1, 3)).reshape(B, S, 1024)


# revision 81
# speedup vs baseline: 16745.6612x; 1.0401x over previous
# Trainium2 Bass kernel for nn_AttentionWithMoLE — 8-core tensor-parallel over heads.
#
# Sharding: core c owns q-heads {2c, 2c+1} (wq cols 128c:128c+128), kv-head c
# (wk/wv cols 64c:64c+64), wo rows 128c:128c+128. Host sums the 8 partial outputs.
#
# MoLE algebra used on device (validated against the reference in numpy):
#   xq = 2*base + sum_e gate_e * lora_e          (softmax gates sum to 1)
#   logits[b,e'] = sum_s r[b,s]*(P1[b,s,e'] - mu[b,s]*C1[s,e']) + C2[e']  (then /tau)
#   P1[b,s,e'] = sum_o base[o]*Wgb[s,o,e'] + sum_er h[er]*W2[s,er,e']
#     where Wgb = sum_e Wgf[s,e,o,e'] (feature-shard) and W2 = Bp-folded Wgf
#     (replicated, /8).  s1 = 4*sum_o base + h.b2 rides as a 5th "e'" column.
#   s2 = sum_f ecat^2 (feature shard), ecat = pl + base broadcast over e.
# Stats ([p,st,b,18]: per proj P1[4]+s1, then s2 x3) AllReduce'd, gates
# computed replicated, then pass B builds q/k/v + RoPE + causal attention
# (scoresT orientation, exp without max-subtraction, bf16 probs, fused rowsum
# via a ones-column appended to V) + output projection (fp16 partials).
import sys
import numpy as np

sys.path.insert(0, '/opt/trn_rl_repo')

import concourse.bass as bass
import concourse.bacc as bacc
import concourse.tile as tile
import concourse.mybir as mybir
from concourse.masks import make_identity

NC = 8
B, S, D, H, KVH, HD, NE, R = 4, 1024, 1024, 16, 8, 64, 4, 16
SCALING, GEPS = 2.0, 1e-6
BS = B * S
NST = S // 128           # 8 s-tiles
NT = NST * B             # 32 token tiles of 128
F16 = mybir.dt.float16
BF16 = mybir.dt.bfloat16
F32 = mybir.dt.float32
AX = mybir.AxisListType
OP = mybir.AluOpType
AF = mybir.ActivationFunctionType

_CACHE = {}
PHASES = 6
APARTS = 7
NOCOLL = False   # timing mode: collectives not supported inside For_i loops


def _bcast_ap(ap, ins):
    """Insert step-0 broadcast dims: ins = list of (pos, count) into ap.ap."""
    dims = [list(d) for d in ap.ap]
    for pos, count in ins:
        dims.insert(pos, [0, count])
    return bass.AP(tensor=ap.tensor, offset=ap.offset, ap=dims)


def _emit(nc, tc, ctx, rep):
    g = nc._kernel_io  # dict of dram handles
    sfx = f"_r{rep}"

    # ---------------- resident pools ----------------
    res = ctx.enter_context(tc.tile_pool(name="res" + sfx, bufs=1))
    pb16_all = res.tile([128, NT, 448], F16)    # [q128|Aq64|k64|Ak64|v64|Av64]
    hqk_all = res.tile([128, NT, 128], F16)     # [er(q0:64,k64:128), t, tok]
    hv_all = res.tile([64, NT, 128], F16)
    xfb_all = res.tile([128, NT, 256], F16)     # 2*base, [q128|k64|v64]
    qT_all = res.tile([64, NST, B, 2, 128], F16)  # [hd, st, b, head, tok] (prescaled 1/8)
    kT_all = res.tile([64, NST, B, 128], F16)
    vaug_all = res.tile([128, NST, B, 65], BF16)
    att_all = res.tile([128, NT, 128], F16)     # [tok(sq), t, o(2 heads)]
    stats_all = res.tile([128, NST, B, 18], F32)
    stats16 = res.tile([128, NST, B, 18], F16)
    stats_ar = res.tile([128, NST, B, 18], F16)
    wqkv_sb = res.tile([128, 8, 451], F16)      # [wq|Aq|wk|Ak|wv|Av|s1cols] per chunk
    blq_sb = res.tile([64, 512], F16)
    blk_sb = res.tile([128, 256], F16)
    blv_sb = res.tile([64, 256], F16)
    bbqk_sb = res.tile([128, 256], F16)
    bbv_sb = res.tile([64, 256], F16)
    wo_sb = res.tile([128, 1024], F16)
    cos_sb = res.tile([128, NST, 96], F32)
    sin_sb = res.tile([128, NST, 96], F32)
    c1_sb = res.tile([128, NST, 12], F32)
    c2it_sb = res.tile([1, 48], F32)
    itau_sb = res.tile([128, 3], F32)
    maskt_sb = res.tile([128, 128], F32)
    ident_sb = res.tile([128, 128], F16)
    ones_sb = res.tile([128, 1], F32)
    geps_sb = res.tile([128, 1], F32)
    zero_sb = res.tile([128, 1], F32)
    gvqk_sb = res.tile([128, B], F32)
    gvv_sb = res.tile([64, B], F32)
    gates_sb = res.tile([1, 48], F16)
    g48_sb = res.tile([48, 1], F16)
    g48x4_sb = res.tile([48, B], F16)
    selqk_sb = res.tile([48, 128], F16)
    selv_sb = res.tile([48, 64], F16)
    bmask_sb = res.tile([48, B], F16)

    dma = nc.sync.dma_start
    dma(out=wqkv_sb[:], in_=g['wqkv16'][:])
    dma(out=blq_sb[:], in_=g['blq16'][:])
    dma(out=blk_sb[64:128, :], in_=g['blk16'][:])
    dma(out=blv_sb[:], in_=g['blv16'][:])
    dma(out=bbqk_sb[:], in_=g['bbqk16'][:])
    dma(out=bbv_sb[:], in_=g['bbv16'][:])
    dma(out=wo_sb[:], in_=g['wo16'][:])
    dma(out=cos_sb[:], in_=g['cosr'][:])
    dma(out=sin_sb[:], in_=g['sinr'][:])
    dma(out=c1_sb[:], in_=g['c1_12'][:])
    c2 = g['c2it48'][:]
    dma(out=c2it_sb[:], in_=bass.AP(tensor=c2.tensor, offset=c2.offset,
                                    ap=[[0, 1], [1, 48]]))
    it = g['itau3'][:]
    dma(out=itau_sb[:], in_=bass.AP(tensor=it.tensor, offset=it.offset,
                                    ap=[[0, 128], [1, 3]]))
    dma(out=maskt_sb[:], in_=g['maskt'][:])
    dma(out=selqk_sb[:], in_=g['selqk'][:])
    dma(out=selv_sb[:], in_=g['selv'][:])
    dma(out=bmask_sb[:], in_=g['bmask'][:])
    make_identity(nc, ident_sb[:])
    nc.vector.memset(ones_sb[:], 1.0)
    nc.vector.memset(geps_sb[:], GEPS)
    nc.vector.memset(zero_sb[:], 0.0)
    nc.vector.memset(vaug_all[:, :, :, 64:65], 1.0)

    # ---------------- phase A ----------------
    if PHASES < 1:
        return
    segs = [(0, 192), (192, 320), (320, 448)]   # per-proj [base|h] blocks
    with tc.tile_pool(name="pa" + sfx, bufs=2) as pa, \
         tc.tile_pool(name="ppb" + sfx, bufs=2, space="PSUM") as ppb, \
         tc.tile_pool(name="ppt" + sfx, bufs=2, space="PSUM") as ppt, \
         tc.tile_pool(name="ppl" + sfx, bufs=2, space="PSUM") as ppl:
        for st in range(NST):
            wcat_t = pa.tile([128, 4, 448], F16, tag="wcat")
            dma(out=wcat_t[:], in_=g['wcat16'][:][st])
            xt = pa.tile([128, B, 8, 128], F16, tag="xt")
            dma(out=xt[:], in_=g['xt16'][:][st])
            for b in range(B):
                t = st * B + b
                pbase = ppb.tile([128, 451], F32, tag="pbase")
                for ch in range(8):
                    nc.tensor.matmul(pbase[:], xt[:, b, ch, :], wqkv_sb[:, ch, :],
                                     start=(ch == 0), stop=(ch == 7))
                pb16 = pb16_all[:, t, :]
                nc.scalar.copy(pb16, pbase[:, 0:448])
                sa = stats_all
                # s1 (exact, via matmul cols) -> stats cols 4, 9, 14
                pbv = pbase[:]
                s1dst = bass.AP(tensor=sa.tensor, offset=sa[:, st, b, 4:5].offset,
                                ap=[list(sa[:, st, b, 4:5].ap[0]), [5, 3]])
                nc.scalar.copy(s1dst, pbase[:, 448:451])
                # er-major h via transposes: hqk [q-h | k-h], hv
                pt = ppt.tile([128, 256], F16, tag="pt")
                nc.tensor.transpose(pt[0:64, 0:128], pb16_all[:, t, 128:192],
                                    ident_sb[:])
                nc.tensor.transpose(pt[64:128, 0:128], pb16_all[:, t, 256:320],
                                    ident_sb[:])
                nc.tensor.transpose(pt[0:64, 128:256], pb16_all[:, t, 384:448],
                                    ident_sb[:])
                nc.scalar.copy(hqk_all[:, t, :], pt[:, 0:128])
                nc.scalar.copy(hv_all[:, t, :], pt[0:64, 128:256])
                if APARTS < 2:
                    continue
                # lora_e via block-diag B' (K=64)
                pl = ppl.tile([128, 1024], F32, tag="pl")
                nc.tensor.matmul(pl[:, 0:512], hqk_all[0:64, t, :], blq_sb[:])
                nc.tensor.matmul(pl[:, 512:768], hqk_all[64:128, t, :], blk_sb[64:128, :])
                nc.tensor.matmul(pl[:, 768:1024], hv_all[:, t, :], blv_sb[:])
                if APARTS < 4:
                    continue
                # ecat = pl + base (broadcast over e); s2 = sum ecat^2 (fused)
                # (DVE reads pl from PSUM directly; Pool is not PSUM-capable)
                ecat = pa.tile([128, 1024], F16, tag="ecat")
                junk = pa.tile([128, 512], F16, tag="junk")
                nc.vector.tensor_add(
                    ecat[:, 0:512].rearrange('p (e o) -> p e o', e=4),
                    pl[:, 0:512].rearrange('p (e o) -> p e o', e=4),
                    _bcast_ap(pb16_all[:, t, 0:128], [(1, 4)]))
                nc.vector.tensor_add(
                    ecat[:, 512:768].rearrange('p (e o) -> p e o', e=4),
                    pl[:, 512:768].rearrange('p (e o) -> p e o', e=4),
                    _bcast_ap(pb16_all[:, t, 192:256], [(1, 4)]))
                nc.vector.tensor_add(
                    ecat[:, 768:1024].rearrange('p (e o) -> p e o', e=4),
                    pl[:, 768:1024].rearrange('p (e o) -> p e o', e=4),
                    _bcast_ap(pb16_all[:, t, 320:384], [(1, 4)]))
                for ci, (f0, f1) in enumerate([(0, 512), (512, 768), (768, 1024)]):
                    nc.scalar.activation(
                        out=junk[:, 0:f1 - f0], in_=ecat[:, f0:f1],
                        func=AF.Square,
                        accum_out=sa[:, st, b, 15 + ci:16 + ci])
                # P1: broadcast mul (Pool) + 3 segmented reduces (DVE)
                wm = pa.tile([128, 4, 448], F16, tag="wm")
                nc.gpsimd.tensor_mul(wm[:], wcat_t[:],
                                     _bcast_ap(pb16_all[:, t, :], [(1, 4)]))
                nc.vector.tensor_reduce(sa[:, st, b, 0:4], wm[:, :, 0:192],
                                        axis=AX.X, op=OP.add)
                nc.vector.tensor_reduce(sa[:, st, b, 5:9], wm[:, :, 192:320],
                                        axis=AX.X, op=OP.add)
                nc.vector.tensor_reduce(sa[:, st, b, 10:14], wm[:, :, 320:448],
                                        axis=AX.X, op=OP.add)

    if PHASES < 2:
        return
    # gate-independent 2*base precompute (overlaps the AllReduce below)
    for t in range(NT):
        nc.vector.tensor_scalar_mul(xfb_all[:, t, 0:128],
                                    pb16_all[:, t, 0:128], 2.0)
        pbv = pb16_all[:, t, 0:1]
        srckv = bass.AP(tensor=pb16_all.tensor, offset=pbv.offset + 192,
                        ap=[list(pbv.ap[0]), [128, 2], [1, 64]])
        nc.vector.tensor_scalar_mul(
            xfb_all[:, t, 128:256].rearrange('p (s o) -> p s o', s=2), srckv, 2.0)
    # ---------------- AllReduce ----------------
    # fp16 collective payload (144KB): stats precision is ample in fp16
    nc.scalar.copy(stats16[:], stats_all[:])
    with tc.tile_pool(name="dr" + sfx, bufs=1, space="DRAM") as dr:
        ar_in = dr.tile([128, NST, B, 18], F16)
        ar_out = dr.tile([128, NST, B, 18], F16)
        dma(out=ar_in[:], in_=stats16[:])
        if NOCOLL:
            dma(out=ar_out[:], in_=ar_in[:])
        else:
            nc.gpsimd.collective_compute(
                "AllReduce", OP.add, replica_groups=[list(range(NC))],
                ins=[ar_in[:]], outs=[ar_out[:]])
        dma(out=stats_ar[:], in_=ar_out[:])

    if PHASES < 3:
        return
    # ---------------- gates (replicated) ----------------
    with tc.tile_pool(name="pg" + sfx, bufs=1) as pg, \
         tc.tile_pool(name="pgp" + sfx, bufs=1, space="PSUM") as pgp:
        lpart = pg.tile([128, 48], F32)
        for pi, Ff in enumerate([NE * H * HD, NE * KVH * HD, NE * KVH * HD]):
            s1 = stats_ar[:, :, :, 5 * pi + 4]
            s2 = stats_ar[:, :, :, 15 + pi]
            mu = pg.tile([128, NST, B], F32, tag=f"mu{pi}")
            vr = pg.tile([128, NST, B], F32, tag=f"vr{pi}")
            rr = pg.tile([128, NST, B], F32, tag=f"rr{pi}")
            rm = pg.tile([128, NST, B], F32, tag=f"rm{pi}")
            t1 = pg.tile([128, NST, B, 4], F32, tag=f"t1{pi}")
            t2 = pg.tile([128, NST, B, 4], F32, tag=f"t2{pi}")
            nc.vector.tensor_scalar_mul(mu[:], s1, 1.0 / Ff)
            nc.vector.tensor_scalar_mul(vr[:], s2, 1.0 / Ff)
            nc.vector.tensor_mul(t2[:, :, :, 0], mu[:], mu[:])
            nc.vector.tensor_sub(vr[:], vr[:], t2[:, :, :, 0])
            nc.scalar.activation(out=rr[:], in_=vr[:], func=AF.Sqrt, bias=geps_sb[:],
                                 scale=1.0)
            nc.vector.reciprocal(rr[:], rr[:])
            nc.vector.tensor_scalar_mul(rr[:], rr[:], itau_sb[:, pi:pi + 1])
            nc.vector.tensor_mul(rm[:], rr[:], mu[:])
            # all 4 experts at once: t1 = P1*rr - C1*rm, reduced over st
            nc.vector.tensor_mul(t1[:], stats_ar[:, :, :, 5 * pi:5 * pi + 4],
                                 _bcast_ap(rr[:], [(3, 4)]))
            nc.vector.tensor_mul(t2[:], _bcast_ap(c1_sb[:, :, 4 * pi:4 * pi + 4],
                                                  [(2, B)]),
                                 _bcast_ap(rm[:], [(3, 4)]))
            nc.vector.tensor_sub(t1[:], t1[:], t2[:])
            src = bass.AP(tensor=t1.tensor, offset=t1[:].offset,
                          ap=[list(t1[:].ap[0]), [4, B], [1, 4], [4 * B, NST]])
            dst = bass.AP(tensor=lpart.tensor, offset=lpart[:].offset + 16 * pi,
                          ap=[list(lpart[:].ap[0]), [4, B], [1, 4]])
            nc.vector.tensor_reduce(dst, src, axis=AX.X, op=OP.add)
        pl48 = pgp.tile([1, 48], F32)
        nc.tensor.matmul(pl48[:], ones_sb[:], lpart[:])
        lg = pg.tile([1, 48], F32)
        nc.vector.tensor_add(lg[:], pl48[:], c2it_sb[:])
        # softmax over e' in groups of 4 (layout (p, b, e'))
        mx = pg.tile([1, 12], F32)
        nc.vector.tensor_reduce(mx[:], lg[:].rearrange('o (g e) -> o g e', e=4),
                                axis=AX.X, op=OP.max)
        for e in range(4):
            nc.vector.tensor_sub(lg[:].rearrange('o (g e) -> o g e', e=4)[:, :, e],
                                 lg[:].rearrange('o (g e) -> o g e', e=4)[:, :, e],
                                 mx[:])
        nc.scalar.activation(out=lg[:], in_=lg[:], func=AF.Exp)
        sm = pg.tile([1, 12], F32)
        nc.vector.tensor_reduce(sm[:], lg[:].rearrange('o (g e) -> o g e', e=4),
                                axis=AX.X, op=OP.add)
        nc.vector.reciprocal(sm[:], sm[:])
        for e in range(4):
            nc.vector.tensor_mul(gates_sb[:].rearrange('o (g e) -> o g e', e=4)[:, :, e],
                                 lg[:].rearrange('o (g e) -> o g e', e=4)[:, :, e],
                                 sm[:])
        # broadcast gates to per-er rows: transpose to 48 partitions, expand
        # per-b columns via mask, then one-hot selection matmuls.
        g48T = pgp.tile([48, 1], F16)
        nc.tensor.transpose(g48T[:], gates_sb[:], ident_sb[0:1, 0:1])
        nc.scalar.copy(g48_sb[:], g48T[:])
        nc.vector.tensor_mul(g48x4_sb[:], bmask_sb[:],
                             _bcast_ap(g48_sb[:, 0], [(1, B)]))
        gvp = pgp.tile([128, B], F32)
        gvvp = pgp.tile([64, B], F32)
        nc.tensor.matmul(gvp[:], selqk_sb[:], g48x4_sb[:])
        nc.tensor.matmul(gvvp[:], selv_sb[:], g48x4_sb[:])
        nc.scalar.copy(gvqk_sb[:], gvp[:])
        nc.scalar.copy(gvv_sb[:], gvvp[:])

    if PHASES < 4:
        return
    # ---------------- phase B (batched over all 8 s-tiles per b) ----------------
    with tc.tile_pool(name="pb" + sfx, bufs=2) as pb, \
         tc.tile_pool(name="ppx" + sfx, bufs=1, space="PSUM") as ppx, \
         tc.tile_pool(name="ppt2" + sfx, bufs=1, space="PSUM") as ppt2:
        for b in range(B):
            # gate-scaled h for all 8 st at once (t stride for fixed b = B*128)
            hgqk = pb.tile([128, NST, 128], F16, tag="hgqk")
            hgv = pb.tile([64, NST, 128], F16, tag="hgv")
            hq0 = hqk_all[:, b, :]
            nc.gpsimd.tensor_scalar_mul(
                hgqk[:], bass.AP(tensor=hqk_all.tensor, offset=hq0.offset,
                                 ap=[list(hq0.ap[0]), [B * 128, NST], [1, 128]]),
                gvqk_sb[:, b:b + 1])
            hv0 = hv_all[0:64, b, :]
            nc.gpsimd.tensor_scalar_mul(
                hgv[:], bass.AP(tensor=hv_all.tensor, offset=hv0.offset,
                                ap=[list(hv0.ap[0]), [B * 128, NST], [1, 128]]),
                gvv_sb[:, b:b + 1])
            px = ppx.tile([128, NST, 256], F32, tag="px")
            for st in range(NST):
                nc.tensor.matmul(px[:, st, :], hgqk[:, st, :], bbqk_sb[:],
                                 start=True, stop=False)
                nc.tensor.matmul(px[:, st, :], hgv[:, st, :], bbv_sb[:],
                                 start=False, stop=True)
            xf = pb.tile([128, NST, 192], F16, tag="xf")
            xfb0 = xfb_all[:, b, :]
            xfbv = bass.AP(tensor=xfb_all.tensor, offset=xfb0.offset,
                           ap=[list(xfb0.ap[0]), [B * 256, NST], [1, 192]])
            nc.vector.tensor_add(
                xf[:], xfbv,
                px[:].rearrange('p st o -> p st o')[:, :, 0:192])
            xfbv2 = bass.AP(tensor=xfb_all.tensor, offset=xfb0.offset + 192,
                            ap=[list(xfb0.ap[0]), [B * 256, NST], [1, 64]])
            nc.vector.tensor_add(
                vaug_all[:, :, b, 0:64], xfbv2,
                px[:].rearrange('p st o -> p st o')[:, :, 192:256])
            # rope for all st: 4-dim APs [p, st, head(3), 32]
            xr = pb.tile([128, NST, 192], F16, tag="xr")
            tmp1 = pb.tile([128, NST, 96], F32, tag="tmp1")
            tmp2 = pb.tile([128, NST, 96], F32, tag="tmp2")
            tmp3 = pb.tile([128, NST, 96], F32, tag="tmp3")
            tmp4 = pb.tile([128, NST, 96], F32, tag="tmp4")
            xfo = xf[:]
            pstep = xfo.ap[0][0]
            xe = bass.AP(tensor=xf.tensor, offset=xfo.offset,
                         ap=[[pstep, 128], [192, NST], [64, 3], [2, 32]])
            xo = bass.AP(tensor=xf.tensor, offset=xfo.offset + 1,
                         ap=[[pstep, 128], [192, NST], [64, 3], [2, 32]])
            xro = xr[:]
            prstep = xro.ap[0][0]
            xre = bass.AP(tensor=xr.tensor, offset=xro.offset,
                          ap=[[prstep, 128], [192, NST], [64, 3], [2, 32]])
            xroo = bass.AP(tensor=xr.tensor, offset=xro.offset + 1,
                           ap=[[prstep, 128], [192, NST], [64, 3], [2, 32]])
            cb = cos_sb[:].rearrange('p st (h i) -> p st h i', h=3)
            sb_ = sin_sb[:].rearrange('p st (h i) -> p st h i', h=3)
            T1 = tmp1[:].rearrange('p st (h i) -> p st h i', h=3)
            T2 = tmp2[:].rearrange('p st (h i) -> p st h i', h=3)
            T3 = tmp3[:].rearrange('p st (h i) -> p st h i', h=3)
            T4 = tmp4[:].rearrange('p st (h i) -> p st h i', h=3)
            nc.vector.tensor_mul(T1, xe, cb)
            nc.vector.tensor_mul(T2, xo, sb_)
            nc.vector.tensor_sub(xre, T1, T2)
            nc.gpsimd.tensor_mul(T3, xe, sb_)
            nc.gpsimd.tensor_mul(T4, xo, cb)
            nc.gpsimd.tensor_add(xroo, T3, T4)
            # transposes into a per-b psum strip (512-col stride keeps each
            # 384-wide transpose inside a psum bank)
            pt = ppt2.tile([64, NST, 512], F16, tag="pt")
            for st in range(NST):
                nc.tensor.transpose(pt[:, st, 0:128], xr[:, st, 0:64], ident_sb[:])
                nc.tensor.transpose(pt[:, st, 128:256], xr[:, st, 64:128],
                                    ident_sb[:])
                nc.tensor.transpose(pt[:, st, 256:384], xr[:, st, 128:192],
                                    ident_sb[:])
            qd0 = qT_all[:, 0, b, :, :]
            nc.scalar.mul(
                bass.AP(tensor=qT_all.tensor, offset=qd0.offset,
                        ap=[list(qd0.ap[0]), [B * 256, NST], [1, 256]]),
                pt[:, :, 0:256], 0.125)
            kd0 = kT_all[:, 0, b, :]
            nc.scalar.copy(
                bass.AP(tensor=kT_all.tensor, offset=kd0.offset,
                        ap=[list(kd0.ap[0]), [B * 128, NST], [1, 128]]),
                pt[:, :, 256:384])

    if PHASES < 5:
        return
    # ---------------- attention ----------------
    with tc.tile_pool(name="pat" + sfx, bufs=2) as pat, \
         tc.tile_pool(name="pps" + sfx, bufs=2, space="PSUM") as pps, \
         tc.tile_pool(name="ppa" + sfx, bufs=2, space="PSUM") as ppa:
        for b in range(B):
            for hh in range(2):
                probs = pat.tile([128, NST, 1024], BF16, tag="probs")
                for i in range(NST):
                    ki = kT_all[:, i, b, :]
                    c0 = i * 128
                    ps = pps.tile([128, 1024], F32, tag="ps")
                    for (m0, m1) in [(c0, 512), (max(512, c0), 1024)]:
                        if m0 >= m1:
                            continue
                        st0 = m0 // 128
                        rhs = qT_all[:, st0:(m1 // 128), b, hh, :]
                        nc.tensor.matmul(ps[:, m0:m1], ki, rhs)
                    nc.vector.tensor_add(ps[:, c0:c0 + 128], ps[:, c0:c0 + 128],
                                         maskt_sb[:])
                    nc.scalar.activation(out=probs[:, i, c0:1024],
                                         in_=ps[:, c0:1024], func=AF.Exp)
                pa8 = ppa.tile([128, NST, 128], F32, tag="pa8")
                for j in range(NST):
                    for i in range(j + 1):
                        nc.tensor.matmul(pa8[:, j, 0:65],
                                         probs[:, i, 128 * j:128 * j + 128],
                                         vaug_all[:, i, b, :],
                                         start=(i == 0), stop=(i == j))
                rc8 = pat.tile([128, NST], F32, tag="rc8")
                pav = pa8[:]
                nc.vector.reciprocal(
                    rc8[:], bass.AP(tensor=pa8.tensor, offset=pav.offset + 64,
                                    ap=[list(pav.ap[0]), [128, NST]]))
                ad0 = att_all[:, b, 64 * hh:64 * hh + 64]
                nc.vector.tensor_mul(
                    bass.AP(tensor=att_all.tensor, offset=ad0.offset,
                            ap=[list(ad0.ap[0]), [B * 128, NST], [1, 64]]),
                    pa8[:, :, 0:64], _bcast_ap(rc8[:], [(2, 64)]))

    if PHASES < 6:
        return
    # ---------------- output projection ----------------
    with tc.tile_pool(name="pw" + sfx, bufs=2) as pw, \
         tc.tile_pool(name="ppo" + sfx, bufs=2, space="PSUM") as ppo, \
         tc.tile_pool(name="ppat" + sfx, bufs=2, space="PSUM") as ppat:
        for b in range(B):
            paT = ppat.tile([128, NST, 128], F16, tag="paT")
            for st in range(NST):
                nc.tensor.transpose(paT[:, st, :], att_all[:, st * B + b, :],
                                    ident_sb[:])
            aT = pw.tile([128, NST, 128], F16, tag="aT")
            nc.scalar.copy(aT[:], paT[:])
            ob = pw.tile([128, NST, 1024], F16, tag="ob")
            for st in range(NST):
                po = ppo.tile([128, 1024], F32, tag="po")
                nc.tensor.matmul(po[:, 0:512], aT[:, st, :], wo_sb[:, 0:512])
                nc.tensor.matmul(po[:, 512:1024], aT[:, st, :], wo_sb[:, 512:1024])
                if st % 2 == 0:
                    nc.scalar.copy(ob[:, st, :], po[:])
                else:
                    nc.vector.tensor_copy(ob[:, st, :], po[:])
            dma(out=g['outp'][:][b], in_=ob[:])


def build_kernel(repeat=1, loopn=0):
    key = (repeat, PHASES, APARTS, loopn, NOCOLL)
    if key in _CACHE:
        return _CACHE[key]
    nc = bacc.Bacc()
    io = {}
    def din(name, shape, dt):
        io[name] = nc.dram_tensor(name, list(shape), dt, kind="ExternalInput")
    din('xt16', (NST, 128, B, 8, 128), F16)
    din('wqkv16', (128, 8, 451), F16)
    din('wcat16', (NST, 128, 4, 448), F16)
    din('blq16', (64, 512), F16)
    din('blk16', (64, 256), F16)
    din('blv16', (64, 256), F16)
    din('bbqk16', (128, 256), F16)
    din('bbv16', (64, 256), F16)
    din('c1_12', (128, NST, 12), F32)
    din('c2it48', (48,), F32)
    din('itau3', (3,), F32)
    din('cosr', (128, NST, 96), F32)
    din('sinr', (128, NST, 96), F32)
    din('maskt', (128, 128), F32)
    din('selqk', (48, 128), F16)
    din('selv', (48, 64), F16)
    din('bmask', (48, B), F16)
    din('wo16', (128, 1024), F16)
    io['outp'] = nc.dram_tensor('outp', [B, 128, NST, 1024], F16,
                                kind="ExternalOutput")
    nc._kernel_io = io
    from contextlib import ExitStack
    with tile.TileContext(nc) as tc:
        if loopn:
            # hardware loop: same body executed loopn times (timing mode)
            with tc.For_i(0, loopn):
                with ExitStack() as ctx:
                    _emit(nc, tc, ctx, 0)
        else:
            for rep in range(repeat):
                with ExitStack() as ctx:
                    _emit(nc, tc, ctx, rep)
    nc.finalize()
    _CACHE[key] = nc
    return nc


def prep_inputs(inputs):
    """Host-side sharding prep: returns in_maps (list of 8 dicts)."""
    f = np.float32
    x = np.asarray(inputs['x'], f)
    # xt16[st, p, b, ch, tk] = x[b, st*128+tk, ch*128+p]
    xr8 = np.asarray(x.transpose(2, 0, 1), np.float16).reshape(8, 128, B, NST, 128)
    xt16 = np.ascontiguousarray(xr8.transpose(3, 1, 2, 0, 4))
    cos3 = np.tile(np.asarray(inputs['cos'], f), (1, 3)).reshape(NST, 128, 96)
    sin3 = np.tile(np.asarray(inputs['sin'], f), (1, 3)).reshape(NST, 128, 96)
    cosr = np.ascontiguousarray(cos3.transpose(1, 0, 2))
    sinr = np.ascontiguousarray(sin3.transpose(1, 0, 2))
    maskt = np.ascontiguousarray(np.asarray(inputs['mask'], f)[0:128, 0:128].T)
    # gate broadcast helpers: gates flat col = pi*16 + b*4 + e
    selqk = np.zeros((48, 128), np.float16)
    for er in range(128):
        pi, e = er // 64, (er % 64) // 16
        for b in range(B):
            selqk[pi * 16 + b * 4 + e, er] = 1.0
    selv = np.zeros((48, 64), np.float16)
    for er in range(64):
        for b in range(B):
            selv[2 * 16 + b * 4 + er // 16, er] = 1.0
    bmask = np.zeros((48, B), np.float16)
    for fl in range(48):
        bmask[fl, (fl % 16) // 4] = 1.0
    in_maps = []
    pr = {}
    for p, Of in [('q', H * HD), ('k', KVH * HD), ('v', KVH * HD)]:
        A = np.asarray(inputs[f'A_{p}'], f)
        Bm = np.asarray(inputs[f'B_{p}'], f)
        gg = np.asarray(inputs[f'g_{p}'], f)
        bb = np.asarray(inputs[f'b_{p}'], f)
        We = np.asarray(inputs[f'We_{p}'], f)
        tau = float(np.asarray(inputs[f'tau_{p}']))
        itau = 1.0 / max(tau, 1e-6)
        Acat = np.ascontiguousarray(A.transpose(1, 0, 2).reshape(D, NE * R))
        Bp = SCALING * Bm                      # [E,R,Of]
        gv = gg.reshape(NE, Of)
        Wgf = We.reshape(S, NE, Of, NE) * gv[None, :, :, None]
        Wgb = Wgf.sum(axis=1)                  # [S, Of, 4]
        W2 = np.einsum('ero,seoE->serE', Bp, Wgf).reshape(S, NE * R, NE) / NC
        b2 = Bp.sum(axis=2).reshape(NE * R) / NC
        C1 = Wgf.sum(axis=(1, 2)) * itau       # [S,4]
        C2 = (We.reshape(S, NE * Of, NE) * bb[None, :, None]).sum((0, 1)) * itau
        pr[p] = dict(Acat=Acat, Bp=Bp, Wgb=Wgb, W2=W2, b2=b2, C1=C1, C2=C2,
                     itau=itau)
    c1_12 = np.ascontiguousarray(
        np.concatenate([pr[p]['C1'] for p in 'qkv'], 1).astype(f)
        .reshape(NST, 128, 12).transpose(1, 0, 2))
    c2it48 = np.zeros(48, f)
    for pi, p in enumerate('qkv'):
        for b in range(B):
            c2it48[pi * 16 + b * 4:pi * 16 + b * 4 + 4] = pr[p]['C2']
    itau3 = np.array([pr[p]['itau'] for p in 'qkv'], f)

    wq = np.asarray(inputs['wq'], f)
    wk = np.asarray(inputs['wk'], f)
    wv = np.asarray(inputs['wv'], f)
    wo = np.asarray(inputs['wo'], f)
    # s1 columns (exact full-feature sums, /NC since every core computes them)
    s1cols = np.stack(
        [(4.0 * Wfull.sum(axis=1) / NC + pr[p]['Acat'] @ pr[p]['b2'])
         for p, Wfull in [('q', wq), ('k', wk), ('v', wv)]], axis=1)  # [D,3]
    for c in range(NC):
        qs = slice(128 * c, 128 * c + 128)
        ks = slice(64 * c, 64 * c + 64)
        # rhs chunks: [wq(128)|Aq(64)|wk(64)|Ak(64)|wv(64)|Av(64)|s1(3)] per ch
        wqkv = np.concatenate(
            [wq[:, qs], pr['q']['Acat'], wk[:, ks], pr['k']['Acat'],
             wv[:, ks], pr['v']['Acat'], s1cols], 1)             # [D,451]
        wqkv16 = np.ascontiguousarray(
            wqkv.reshape(8, 128, 451).transpose(1, 0, 2)).astype(np.float16)
        # wcat [S -> (st,p), 4, 448]
        wcat = np.zeros((S, 4, 448), f)
        for pi_, (p, sh, o0) in enumerate([('q', qs, 0), ('k', ks, 192),
                                           ('v', ks, 320)]):
            wcat[:, :, o0:o0 + (128 if p == 'q' else 64)] = \
                pr[p]['Wgb'][:, sh, :].transpose(0, 2, 1)
            h0 = o0 + (128 if p == 'q' else 64)
            wcat[:, :, h0:h0 + 64] = pr[p]['W2'].transpose(0, 2, 1)
        wcat16 = np.ascontiguousarray(
            wcat.reshape(NST, 128, 4, 448)).astype(np.float16)
        m = dict(xt16=xt16, wqkv16=wqkv16, wcat16=wcat16, c1_12=c1_12,
                 c2it48=c2it48, itau3=itau3, cosr=cosr, sinr=sinr, maskt=maskt,
                 selqk=selqk, selv=selv, bmask=bmask)
        # lora block matrices
        blq = np.zeros((64, 512), f)
        blk = np.zeros((64, 256), f)
        blv = np.zeros((64, 256), f)
        for e in range(NE):
            blq[e * 16:e * 16 + 16, e * 128:e * 128 + 128] = pr['q']['Bp'][e][:, qs]
            blk[e * 16:e * 16 + 16, e * 64:e * 64 + 64] = pr['k']['Bp'][e][:, ks]
            blv[e * 16:e * 16 + 16, e * 64:e * 64 + 64] = pr['v']['Bp'][e][:, ks]
        bbqk = np.zeros((128, 256), f)
        bbv = np.zeros((64, 256), f)
        bbqk[0:64, 0:128] = pr['q']['Bp'][:, :, qs].reshape(64, 128)
        bbqk[64:128, 128:192] = pr['k']['Bp'][:, :, ks].reshape(64, 64)
        bbv[:, 192:256] = pr['v']['Bp'][:, :, ks].reshape(64, 64)
        m['blq16'] = blq.astype(np.float16)
        m['blk16'] = blk.astype(np.float16)
        m['blv16'] = blv.astype(np.float16)
        m['bbqk16'] = bbqk.astype(np.float16)
        m['bbv16'] = bbv.astype(np.float16)
        m['wo16'] = wo[qs, :].astype(np.float16)
        in_maps.append(m)
    return in_maps


def run_on_device(in_maps, repeat=1, loopn=0):
    from concourse.bass_utils import run_bass_kernel_spmd
    nc = build_kernel(repeat, loopn)
    res = run_bass_kernel_spmd(nc, in_maps, list(range(NC)))
    return res


def _run_sim(in_maps):
    from concourse.bass_interp import MultiCoreSim
    nc = build_kernel(1)
    sim = MultiCoreSim(nc, NC, num_workers=NC)
    for c in range(NC):
        for name, arr in in_maps[c].items():
            sim.cores[c].tensor(name)[:] = arr
    sim.simulate()
    return [{'outp': np.asarray(sim.cores[c].tensor('outp'))} for c in range(NC)]


def kernel(**inputs):
    in_maps = prep_inputs(inputs)
    try:
        results = run_on_device(in_maps, repeat=1).results
    except Exception as e:
        sys.stderr.write(f"device run failed ({e}); falling back to CoreSim\n")
        results = _run_sim(in_maps)
    out = np.zeros((B, 128, NST, 1024), np.float32)
    for c in range(NC):
        out += np.asarray(results[c]['outp'], np.float32)
    return np.ascontiguousarray(out.transpose(0, 2, 1, 3)).reshape(B, S, 1024)
